# revision 35
# baseline (speedup 1.0000x reference)
"""Bahdanau additive attention kernel for Trainium2 (Bass/Tile).

Shapes (hardcoded from the problem spec):
  encoder_outputs [8, 256, 512] f32, decoder_outputs [8, 128, 512] f32,
  encoder_mask    [8, 256] bool,  W1/W2 [512, 512] f32,  V [512, 1] f32.

Sharding: data-parallel over batch B=8 across the 8 NeuronCores; the
W1/W2/V weights are replicated.  Each core computes one batch element:
  ew = enc @ W1; dw = dec @ W2
  score[t,s] = sum_h V[h] * tanh(ew[s,h] + dw[t,h]) + mask_bias[s]
  attn = softmax_s(score); ctx = attn @ enc

Key algorithmic point: masked encoder positions get attention weight
EXACTLY 0.0 in the reference (exp(score - 1e9 - max) underflows), so the
expensive tanh volume only needs the unmasked positions.  The kernel
gathers the ~50% unmasked encoder rows (indices prepared host-side as
sharding metadata, gather itself via on-device indirect DMA), runs the
whole pipeline on S_PAD <= 256 compacted positions, and scatters the
attention row back to the full 256 columns with a one-hot matmul.

Engine split per core (the Td*S_PAD*H element inner part):
  - DVE + GPSIMD: build X[h, (t,c,u)] = ew_T + dw_T[:, t] via
    per-partition-scalar adds (2x DVE mode; 1/8 of tiles on GPSIMD)
  - ACT: tanh on [128, TG*4*S_PAD] tiles (amortizes the per-instruction
    SBUF access bubble; ACT is the throughput floor of this problem)
  - PE : score accumulation via one-hot-padded V matmuls (M=32 strips),
    mask add as a K=1 broadcast matmul, projections/transposes/context,
    and the final scatter matmul
"""

from contextlib import ExitStack

import numpy as np

import concourse.bass as bass
import concourse.tile as tile
from concourse import bacc, bass_utils, mybir
from concourse._compat import with_exitstack
from concourse.masks import make_identity

B, TD, TE, H = 8, 128, 256, 512
P = 128
HC = H // P  # 4 h-chunks
TG = 8  # t's per ACT group
F32 = mybir.dt.float32
F16 = mybir.dt.float16
I32 = mybir.dt.int32


@with_exitstack
def _attention_kernel(ctx: ExitStack, tc: tile.TileContext, S, enc, dec, idx, sprime,
                      w1, w2, v, ctx_out, attn_out):
    nc = tc.nc
    add = mybir.AluOpType.add
    mult = mybir.AluOpType.mult

    # u-chunks of the compacted encoder axis (partition-dim blocks)
    uchunks = [(0, min(S, P))] + ([(P, S - P)] if S > P else [])

    singles = ctx.enter_context(tc.tile_pool(name="singles", bufs=1))
    xpool = ctx.enter_context(tc.tile_pool(name="xpool", bufs=3))
    ypool = ctx.enter_context(tc.tile_pool(name="ypool", bufs=3))
    psum = ctx.enter_context(tc.tile_pool(name="psum", bufs=3, space="PSUM"))
    score_pool = ctx.enter_context(tc.tile_pool(name="score", bufs=1, space="PSUM"))
    ctx_psum_pool = ctx.enter_context(tc.tile_pool(name="ctxp", bufs=1, space="PSUM"))

    # ---------------- input-independent constants first ----------------
    # (make_identity ends with a GPSIMD drain that would otherwise serialize
    # behind any DMA already queued on the gpsimd engine)
    ident = singles.tile([P, P], F32)
    make_identity(nc, ident)
    iota_i = singles.tile([P, TE], I32)  # 0..255 in every partition row
    nc.gpsimd.iota(iota_i, pattern=[[1, TE]], base=0, channel_multiplier=0)
    iota_f = singles.tile([P, TE], F32)
    nc.vector.tensor_copy(out=iota_f, in_=iota_i)
    ones_row = singles.tile([1, P], F16)
    nc.vector.memset(ones_row, 1.0)
    # PE warmup during the DMA wait: dummy matmuls keep the PE busy past the
    # HAM activity window so the transposes/projections run at 2.4 GHz.
    warm_sb = singles.tile([P, P], F16)
    nc.vector.memset(warm_sb, 0.0)
    warm_ps = psum.tile([P, P], F32, tag="tr")
    for _ in range(14):
        nc.tensor.matmul(warm_ps, lhsT=warm_sb, rhs=warm_sb, start=True, stop=True)

    # ---------------- loads ----------------
    # Each DMA queue sustains only ~95 GB/s, so the ~2.3MB of inputs are
    # split across the three dispatch engines (sync/scalar/gpsimd), ordered
    # by when the consumer needs them: idx/dec -> enc gather/W2 -> W1.
    idx_sb = singles.tile([P, 2], I32)  # column ci holds idx[ci*128 + p]
    idx_r = idx.rearrange("(u o) -> u o", o=1)
    for (u0, ulen) in uchunks:
        nc.scalar.dma_start(out=idx_sb[:ulen, u0 // P:u0 // P + 1],
                            in_=idx_r[u0:u0 + ulen])
    sprime_sb = singles.tile([1, 1], F32)
    nc.scalar.dma_start(out=sprime_sb, in_=sprime.rearrange("(u o) -> u o", o=1))
    v_sb32 = singles.tile([P, HC], F32)  # [p, c] = V[c*128+p, 0]
    nc.scalar.dma_start(out=v_sb32, in_=v.rearrange("(c p) o -> p (c o)", p=P))
    dec_sb = singles.tile([P, H], F32)
    nc.sync.dma_start(out=dec_sb, in_=dec)
    # gathered encoder rows: enc_u[j] = enc[idx[j]]
    enc_u1 = singles.tile([P, H], F32)
    _gather_last = nc.gpsimd.indirect_dma_start(
        out=enc_u1[:uchunks[0][1], :], out_offset=None, in_=enc[:],
        in_offset=bass.IndirectOffsetOnAxis(ap=idx_sb[:uchunks[0][1], 0:1], axis=0))
    if len(uchunks) > 1:
        enc_u2 = singles.tile([P, H], F32)
        _gather_last = nc.gpsimd.indirect_dma_start(
            out=enc_u2[:uchunks[1][1], :], out_offset=None, in_=enc[:],
            in_offset=bass.IndirectOffsetOnAxis(
                ap=idx_sb[:uchunks[1][1], 1:2], axis=0))
    w1_sb = singles.tile([P, HC, H], F32)  # [p, hb, k] = W1[hb*128+p, k]
    w1_r = w1.rearrange("(hb p) k -> p hb k", p=P)
    nc.sync.dma_start(out=w1_sb[:, 0:2], in_=w1_r[:, 0:2])
    nc.scalar.dma_start(out=w1_sb[:, 2:4], in_=w1_r[:, 2:4])
    w2_sb = singles.tile([P, HC, H], F32)
    w2_r = w2.rearrange("(hb p) k -> p hb k", p=P)
    nc.scalar.dma_start(out=w2_sb[:, 0:2], in_=w2_r[:, 0:2])
    nc.sync.dma_start(out=w2_sb[:, 2:4], in_=w2_r[:, 2:4])
    # fp16 copies of the weights: fp32 matmuls lower to two PE passes, so
    # halving the projection dtype halves the critical-path PE work.  The
    # casts run on the otherwise-idle prologue DVE, one per DMA half.
    w1_16 = singles.tile([P, HC, H], F16)
    w2_16 = singles.tile([P, HC, H], F16)
    for hb in range(HC):
        nc.vector.tensor_copy(out=w2_16[:, hb], in_=w2_sb[:, hb])
        nc.vector.tensor_copy(out=w1_16[:, hb], in_=w1_sb[:, hb])

    # permuted-column mask bias row: 0 for u < S', -60000 beyond (-1e9 like
    # the reference would overflow fp16; any bias <= -1e3 gives
    # exp(score + bias) == 0.0 exactly in fp32, matching the reference).
    # Built on DVE: the mask matmul is the PSUM start=True writer that every
    # V-matmul waits on, so it must not sit behind slow GPSIMD ops.
    mask_bias = singles.tile([1, TE], F16)
    nc.vector.tensor_scalar(out=mask_bias, in0=iota_f[0:1, :],
                            scalar1=sprime_sb[0:1, 0:1], scalar2=-6e4,
                            op0=mybir.AluOpType.is_ge, op1=mult)

    # ---------------- transposes + projections ----------------
    # dec chain first: dec_T [h, t] then dw_T [k, t] (the X-build scalars),
    # then the enc chain: enc_T [h, u] and ew_T [k, u].
    dec_T = singles.tile([P, HC, TD], F16)
    for hb in range(HC):
        pt = psum.tile([P, P], F32, tag="tr")
        nc.tensor.transpose(pt, dec_sb[:, hb * P:(hb + 1) * P], ident)
        (nc.vector.tensor_copy(out=dec_T[:, hb, :], in_=pt) if hb % 2 == 0
         else nc.scalar.copy(out=dec_T[:, hb, :], in_=pt))
    dw_T = singles.tile([P, HC, TD], F32)
    for kc in range(HC):
        pdw = psum.tile([P, TD], F32, tag="proj")
        for hb in range(HC):
            nc.tensor.matmul(pdw, lhsT=w2_16[:, hb, kc * P:(kc + 1) * P],
                             rhs=dec_T[:, hb, :], start=(hb == 0), stop=(hb == HC - 1))
        (nc.vector.tensor_copy(out=dw_T[:, kc, :], in_=pdw) if kc % 2 == 0
         else nc.scalar.copy(out=dw_T[:, kc, :], in_=pdw))
    enc_T = singles.tile([P, HC, S], F16)  # [p, hb, u] = enc[idx[u], hb*128+p]
    for (u0, ulen) in uchunks:
        src = enc_u1 if u0 == 0 else enc_u2
        for hb in range(HC):
            pt = psum.tile([P, P], F32, tag="tr")
            nc.tensor.transpose(pt[:, :ulen], src[:ulen, hb * P:(hb + 1) * P],
                                ident[:ulen, :ulen])
            (nc.vector.tensor_copy(out=enc_T[:, hb, u0:u0 + ulen], in_=pt[:, :ulen])
             if hb % 2 == 0 else
             nc.scalar.copy(out=enc_T[:, hb, u0:u0 + ulen], in_=pt[:, :ulen]))
    ew_T = singles.tile([P, HC, S], F16)  # [p, kc, u] = ew[idx[u], kc*128+p]
    for kc in range(HC):
        pew = psum.tile([P, S], F32, tag="proj")
        for hb in range(HC):
            nc.tensor.matmul(pew, lhsT=w1_16[:, hb, kc * P:(kc + 1) * P],
                             rhs=enc_T[:, hb, :], start=(hb == 0), stop=(hb == HC - 1))
        (nc.vector.tensor_copy(out=ew_T[:, kc, :], in_=pew) if kc % 2 == 0
         else nc.scalar.copy(out=ew_T[:, kc, :], in_=pew))

    # ---------------- derived constants ----------------
    # Built on the otherwise-idle GPSIMD engine and emitted after the
    # projections so they never head-of-line block the in-order DVE stream
    # (they are only needed once the first V-matmuls run, ~20us later).
    # V32[:, c, j, :] is a [128, 32] one-hot-column stationary operand:
    # column j holds V chunk c, all other columns zero.  A matmul with it
    # accumulates V_c . Y into score row (strip_base + j) while adding zero
    # to the other 31 rows of the strip (PSUM out must be 32-row aligned).
    v32 = singles.tile([P, HC, 32, 32], F16)
    _v32ms = nc.gpsimd.memset(v32, 0.0)
    # keep the dep-free memset from being hoisted in front of the gather
    # dispatches on the in-order gpsimd engine
    bass._add_dep_helper(_v32ms.ins, _gather_last.ins, sync=False,
                         reason="order v32 memset after enc gather dispatch")
    for c in range(HC):
        v32c = v32[:, c]
        diag = bass.AP(tensor=v32c.tensor, offset=v32c.offset,
                       ap=[v32c.ap[0], [33, 32]])
        nc.gpsimd.tensor_scalar_add(out=diag, in0=diag, scalar1=v_sb32[:, c:c + 1])

    # scatter matrix G[u, s] = 1 iff idx[u] == s (padding rows hit s=0 but
    # carry exactly-0 attention, so they contribute nothing)
    idx_f = singles.tile([P, 2], F32)
    for (u0, ulen) in uchunks:
        ci = u0 // P
        nc.gpsimd.tensor_copy(out=idx_f[:ulen, ci:ci + 1],
                              in_=idx_sb[:ulen, ci:ci + 1])
    G = singles.tile([P, 2, TE], F32)
    for (u0, ulen) in uchunks:
        nc.gpsimd.tensor_scalar(out=G[:ulen, u0 // P, :], in0=iota_f[:ulen, :],
                                scalar1=idx_f[:ulen, u0 // P:u0 // P + 1], scalar2=None,
                                op0=mybir.AluOpType.is_equal)

    # ---------------- score accumulation ----------------
    score_ps = score_pool.tile([P, TE], F32)  # [t, u]
    # mask bias broadcast into every row t: ones[1,128].T @ mask_bias[1,256]
    nc.tensor.matmul(score_ps, lhsT=ones_row, rhs=mask_bias, start=True, stop=False,
                     skip_group_check=True)

    # Group-size schedule: small first groups let the ACT stream start as
    # soon as possible (less exposed X-build latency); small last groups
    # shorten the final tanh->V-matmul burst before the softmax.
    group_sizes = [2, 6] + [TG] * 14 + [4, 2, 2]
    assert sum(group_sizes) == TD
    t0g = 0
    for gi, tg in enumerate(group_sizes):
        X = xpool.tile([P, TG * HC * S], F16)
        Y = ypool.tile([P, TG * HC * S], F16)
        for tl in range(tg - 1):
            t = t0g + tl
            for c in range(HC):
                j = tl * HC + c
                nc.vector.tensor_scalar(out=X[:, j * S:(j + 1) * S],
                                        in0=ew_T[:, c, :],
                                        scalar1=dw_T[:, c, t:t + 1],
                                        scalar2=None, op0=add)
        # The last t of each group is computed as four fused bias-tanh
        # activations on ScalarE (no X traffic, no DVE work) -- this
        # balances DVE's 2x-mode add rate against ACT's tanh rate.
        t = t0g + tg - 1
        for c in range(HC):
            j = (tg - 1) * HC + c
            nc.scalar.activation(out=Y[:, j * S:(j + 1) * S], in_=ew_T[:, c, :],
                                 func=mybir.ActivationFunctionType.Tanh,
                                 bias=dw_T[:, c, t:t + 1], scale=1.0)
        nc.scalar.activation(out=Y[:, :(tg - 1) * HC * S],
                             in_=X[:, :(tg - 1) * HC * S],
                             func=mybir.ActivationFunctionType.Tanh)
        for tl in range(tg):
            t = t0g + tl
            strip = (t // 32) * 32
            jj = t % 32
            for c in range(HC):
                j = tl * HC + c
                last = (t == TD - 1) and (c == HC - 1)
                nc.tensor.matmul(score_ps[strip:strip + 32, 0:S],
                                 lhsT=v32[:, c, jj, :],
                                 rhs=Y[:, j * S:(j + 1) * S], start=False, stop=last,
                                 skip_group_check=True, tile_position=(0, strip))
        t0g += tg

    # ---------------- softmax over u (no max-subtraction needed:
    # |score| <= sum|V| ~ 16, exp fits easily in fp32) ----------------
    p_sb = singles.tile([P, TE], F32)
    nc.scalar.activation(out=p_sb, in_=score_ps, func=mybir.ActivationFunctionType.Exp)
    den = singles.tile([P, 1], F32)
    nc.vector.tensor_reduce(out=den, in_=p_sb, axis=mybir.AxisListType.X, op=add)
    rec = singles.tile([P, 1], F32)
    nc.vector.reciprocal(out=rec, in_=den)
    attn_sb = singles.tile([P, TE], F32)  # permuted columns
    nc.vector.tensor_scalar(out=attn_sb, in0=p_sb, scalar1=rec[:, 0:1], scalar2=None,
                            op0=mult)

    # ---------------- context + attention scatter ----------------
    # attn_T [u, t] feeds both the context matmul (rhs = gathered enc) and
    # the scatter matmul (rhs = one-hot G) that un-permutes the columns.
    attn_T = singles.tile([P, 2, P], F32)
    for (u0, ulen) in uchunks:
        pt = psum.tile([P, P], F32, tag="tr")
        nc.tensor.transpose(pt[:ulen, :], attn_sb[:, u0:u0 + ulen], ident)
        nc.vector.tensor_copy(out=attn_T[:ulen, u0 // P, :], in_=pt[:ulen, :])
    ctx_ps = ctx_psum_pool.tile([P, H], F32)
    for ci, (u0, ulen) in enumerate(uchunks):
        src = enc_u1 if u0 == 0 else enc_u2
        nc.tensor.matmul(ctx_ps, lhsT=attn_T[:ulen, u0 // P, :], rhs=src[:ulen, :],
                         start=(ci == 0), stop=(ci == len(uchunks) - 1))
    ctx_sb = singles.tile([P, H], F32)
    nc.vector.tensor_copy(out=ctx_sb, in_=ctx_ps)
    nc.sync.dma_start(out=ctx_out, in_=ctx_sb)
    attn_ps = psum.tile([P, TE], F32, tag="proj")
    for ci, (u0, ulen) in enumerate(uchunks):
        nc.tensor.matmul(attn_ps, lhsT=attn_T[:ulen, u0 // P, :],
                         rhs=G[:ulen, u0 // P, :],
                         start=(ci == 0), stop=(ci == len(uchunks) - 1))
    attn_full = singles.tile([P, TE], F32)
    nc.vector.tensor_copy(out=attn_full, in_=attn_ps)
    nc.sync.dma_start(out=attn_out, in_=attn_full)


def build(S):
    nc = bacc.Bacc("TRN2", target_bir_lowering=False, debug=False, num_devices=B)
    enc = nc.dram_tensor("enc", (TE, H), F32, kind="ExternalInput").ap()
    dec = nc.dram_tensor("dec", (TD, H), F32, kind="ExternalInput").ap()
    idx = nc.dram_tensor("idx", (S,), I32, kind="ExternalInput").ap()
    sprime = nc.dram_tensor("sprime", (1,), F32, kind="ExternalInput").ap()
    w1 = nc.dram_tensor("w1", (H, H), F32, kind="ExternalInput").ap()
    w2 = nc.dram_tensor("w2", (H, H), F32, kind="ExternalInput").ap()
    v = nc.dram_tensor("v", (H, 1), F32, kind="ExternalInput").ap()
    ctx_out = nc.dram_tensor("ctx_out", (TD, H), F32, kind="ExternalOutput").ap()
    attn_out = nc.dram_tensor("attn_out", (TD, TE), F32, kind="ExternalOutput").ap()
    with tile.TileContext(nc) as tc:
        _attention_kernel(tc, S, enc, dec, idx, sprime, w1, w2, v, ctx_out, attn_out)
    nc.compile()
    return nc


_NC_CACHE = {}


def _get_nc(S):
    if S not in _NC_CACHE:
        _NC_CACHE[S] = build(S)
    return _NC_CACHE[S]


def make_in_maps(encoder_outputs, decoder_outputs, encoder_mask, W1, W2, V):
    enc = np.ascontiguousarray(np.asarray(encoder_outputs, dtype=np.float32))
    dec = np.ascontiguousarray(np.asarray(decoder_outputs, dtype=np.float32))
    msk = np.asarray(encoder_mask).astype(bool)
    w1 = np.ascontiguousarray(np.asarray(W1, dtype=np.float32))
    w2 = np.ascontiguousarray(np.asarray(W2, dtype=np.float32))
    v = np.ascontiguousarray(np.asarray(V, dtype=np.float32))
    us = [np.nonzero(msk[b])[0].astype(np.int32) for b in range(B)]
    max_sp = max(max((len(u) for u in us), default=1), 1)
    S = min(TE, ((max_sp + 15) // 16) * 16)
    in_maps = []
    for b in range(B):
        idx = np.zeros(S, np.int32)
        idx[:len(us[b])] = us[b]
        in_maps.append({
            "enc": enc[b], "dec": dec[b], "idx": idx,
            "sprime": np.array([len(us[b])], np.float32),
            "w1": w1, "w2": w2, "v": v,
        })
    return in_maps, S


def kernel(encoder_outputs, decoder_outputs, encoder_mask, W1, W2, V, **run_kwargs):
    in_maps, S = make_in_maps(encoder_outputs, decoder_outputs, encoder_mask, W1, W2, V)
    nc = _get_nc(S)
    res = bass_utils.run_bass_kernel_spmd(nc, in_maps, core_ids=list(range(B)),
                                          **run_kwargs)
    ctx = np.stack([res.results[b]["ctx_out"] for b in range(B)])
    attn = np.stack([res.results[b]["attn_out"] for b in range(B)])
    return ctx, attn


# revision 36
# speedup vs baseline: 1.0109x; 1.0109x over previous
"""Bahdanau additive attention kernel for Trainium2 (Bass/Tile).

Shapes (hardcoded from the problem spec):
  encoder_outputs [8, 256, 512] f32, decoder_outputs [8, 128, 512] f32,
  encoder_mask    [8, 256] bool,  W1/W2 [512, 512] f32,  V [512, 1] f32.

Sharding: data-parallel over batch B=8 across the 8 NeuronCores; the
W1/W2/V weights are replicated.  Each core computes one batch element:
  ew = enc @ W1; dw = dec @ W2
  score[t,s] = sum_h V[h] * tanh(ew[s,h] + dw[t,h]) + mask_bias[s]
  attn = softmax_s(score); ctx = attn @ enc

Key algorithmic point: masked encoder positions get attention weight
EXACTLY 0.0 in the reference (exp(score - 1e9 - max) underflows), so the
expensive tanh volume only needs the unmasked positions.  The kernel
gathers the ~50% unmasked encoder rows (indices prepared host-side as
sharding metadata, gather itself via on-device indirect DMA), runs the
whole pipeline on S_PAD <= 256 compacted positions, and scatters the
attention row back to the full 256 columns with a one-hot matmul.

Engine split per core (the Td*S_PAD*H element inner part):
  - DVE + GPSIMD: build X[h, (t,c,u)] = ew_T + dw_T[:, t] via
    per-partition-scalar adds (2x DVE mode; 1/8 of tiles on GPSIMD)
  - ACT: tanh on [128, TG*4*S_PAD] tiles (amortizes the per-instruction
    SBUF access bubble; ACT is the throughput floor of this problem)
  - PE : score accumulation via one-hot-padded V matmuls (M=32 strips),
    mask add as a K=1 broadcast matmul, projections/transposes/context,
    and the final scatter matmul
"""

from contextlib import ExitStack

import numpy as np

import concourse.bass as bass
import concourse.tile as tile
from concourse import bacc, bass_utils, mybir
from concourse._compat import with_exitstack
from concourse.masks import make_identity

B, TD, TE, H = 8, 128, 256, 512
P = 128
HC = H // P  # 4 h-chunks
TG = 8  # t's per ACT group
F32 = mybir.dt.float32
F16 = mybir.dt.float16
I32 = mybir.dt.int32


@with_exitstack
def _attention_kernel(ctx: ExitStack, tc: tile.TileContext, S, enc, dec, idx, sprime,
                      w1, w2, v, ctx_out, attn_out):
    nc = tc.nc
    add = mybir.AluOpType.add
    mult = mybir.AluOpType.mult

    # u-chunks of the compacted encoder axis (partition-dim blocks)
    uchunks = [(0, min(S, P))] + ([(P, S - P)] if S > P else [])

    singles = ctx.enter_context(tc.tile_pool(name="singles", bufs=1))
    xpool = ctx.enter_context(tc.tile_pool(name="xpool", bufs=3))
    ypool = ctx.enter_context(tc.tile_pool(name="ypool", bufs=3))
    psum = ctx.enter_context(tc.tile_pool(name="psum", bufs=3, space="PSUM"))
    score_pool = ctx.enter_context(tc.tile_pool(name="score", bufs=1, space="PSUM"))
    ctx_psum_pool = ctx.enter_context(tc.tile_pool(name="ctxp", bufs=1, space="PSUM"))

    # ---------------- input-independent constants first ----------------
    # (make_identity ends with a GPSIMD drain that would otherwise serialize
    # behind any DMA already queued on the gpsimd engine)
    ident = singles.tile([P, P], F32)
    make_identity(nc, ident)
    iota_i = singles.tile([P, TE], I32)  # 0..255 in every partition row
    nc.gpsimd.iota(iota_i, pattern=[[1, TE]], base=0, channel_multiplier=0)
    iota_f = singles.tile([P, TE], F32)
    nc.vector.tensor_copy(out=iota_f, in_=iota_i)
    ones_row = singles.tile([1, P], F16)
    nc.vector.memset(ones_row, 1.0)
    # PE warmup during the DMA wait: dummy matmuls keep the PE busy past the
    # HAM activity window so the transposes/projections run at 2.4 GHz.
    warm_sb = singles.tile([P, P], F16)
    nc.vector.memset(warm_sb, 0.0)
    warm_ps = psum.tile([P, P], F32, tag="tr")
    for _ in range(14):
        nc.tensor.matmul(warm_ps, lhsT=warm_sb, rhs=warm_sb, start=True, stop=True)

    # ---------------- loads ----------------
    # Each DMA queue sustains only ~95 GB/s, so the ~2.3MB of inputs are
    # split across the three dispatch engines (sync/scalar/gpsimd), ordered
    # by when the consumer needs them: idx/dec -> enc gather/W2 -> W1.
    idx_sb = singles.tile([P, 2], I32)  # column ci holds idx[ci*128 + p]
    idx_r = idx.rearrange("(u o) -> u o", o=1)
    for (u0, ulen) in uchunks:
        nc.scalar.dma_start(out=idx_sb[:ulen, u0 // P:u0 // P + 1],
                            in_=idx_r[u0:u0 + ulen])
    sprime_sb = singles.tile([1, 1], F32)
    nc.scalar.dma_start(out=sprime_sb, in_=sprime.rearrange("(u o) -> u o", o=1))
    v_sb32 = singles.tile([P, HC], F32)  # [p, c] = V[c*128+p, 0]
    nc.scalar.dma_start(out=v_sb32, in_=v.rearrange("(c p) o -> p (c o)", p=P))
    dec_sb = singles.tile([P, H], F32)
    nc.sync.dma_start(out=dec_sb, in_=dec)
    # gathered encoder rows: enc_u[j] = enc[idx[j]]
    enc_u1 = singles.tile([P, H], F32)
    _gather_last = nc.gpsimd.indirect_dma_start(
        out=enc_u1[:uchunks[0][1], :], out_offset=None, in_=enc[:],
        in_offset=bass.IndirectOffsetOnAxis(ap=idx_sb[:uchunks[0][1], 0:1], axis=0))
    if len(uchunks) > 1:
        enc_u2 = singles.tile([P, H], F32)
        _gather_last = nc.gpsimd.indirect_dma_start(
            out=enc_u2[:uchunks[1][1], :], out_offset=None, in_=enc[:],
            in_offset=bass.IndirectOffsetOnAxis(
                ap=idx_sb[:uchunks[1][1], 1:2], axis=0))
    w1_sb = singles.tile([P, HC, H], F32)  # [p, hb, k] = W1[hb*128+p, k]
    w1_r = w1.rearrange("(hb p) k -> p hb k", p=P)
    nc.sync.dma_start(out=w1_sb[:, 0:2], in_=w1_r[:, 0:2])
    nc.scalar.dma_start(out=w1_sb[:, 2:4], in_=w1_r[:, 2:4])
    w2_sb = singles.tile([P, HC, H], F32)
    w2_r = w2.rearrange("(hb p) k -> p hb k", p=P)
    nc.scalar.dma_start(out=w2_sb[:, 0:2], in_=w2_r[:, 0:2])
    nc.sync.dma_start(out=w2_sb[:, 2:4], in_=w2_r[:, 2:4])
    # fp16 copies of the weights: fp32 matmuls lower to two PE passes, so
    # halving the projection dtype halves the critical-path PE work.  The
    # casts run on the otherwise-idle prologue DVE, one per DMA half.
    w1_16 = singles.tile([P, HC, H], F16)
    w2_16 = singles.tile([P, HC, H], F16)
    for hb in range(HC):
        nc.vector.tensor_copy(out=w2_16[:, hb], in_=w2_sb[:, hb])
        nc.vector.tensor_copy(out=w1_16[:, hb], in_=w1_sb[:, hb])

    # permuted-column mask bias row: 0 for u < S', -60000 beyond (-1e9 like
    # the reference would overflow fp16; any bias <= -1e3 gives
    # exp(score + bias) == 0.0 exactly in fp32, matching the reference).
    # Built on DVE: the mask matmul is the PSUM start=True writer that every
    # V-matmul waits on, so it must not sit behind slow GPSIMD ops.
    mask_bias = singles.tile([1, TE], F16)
    nc.vector.tensor_scalar(out=mask_bias, in0=iota_f[0:1, :],
                            scalar1=sprime_sb[0:1, 0:1], scalar2=-6e4,
                            op0=mybir.AluOpType.is_ge, op1=mult)

    # ---------------- transposes + projections ----------------
    # dec chain first: dec_T [h, t] then dw_T [k, t] (the X-build scalars),
    # then the enc chain: enc_T [h, u] and ew_T [k, u].
    dec_T = singles.tile([P, HC, TD], F16)
    for hb in range(HC):
        pt = psum.tile([P, P], F32, tag="tr")
        nc.tensor.transpose(pt, dec_sb[:, hb * P:(hb + 1) * P], ident)
        (nc.vector.tensor_copy(out=dec_T[:, hb, :], in_=pt) if hb % 2 == 0
         else nc.scalar.copy(out=dec_T[:, hb, :], in_=pt))
    dw_T = singles.tile([P, HC, TD], F32)
    for kc in range(HC):
        pdw = psum.tile([P, TD], F32, tag="proj")
        for hb in range(HC):
            nc.tensor.matmul(pdw, lhsT=w2_16[:, hb, kc * P:(kc + 1) * P],
                             rhs=dec_T[:, hb, :], start=(hb == 0), stop=(hb == HC - 1))
        (nc.vector.tensor_copy(out=dw_T[:, kc, :], in_=pdw) if kc % 2 == 0
         else nc.scalar.copy(out=dw_T[:, kc, :], in_=pdw))
    enc_T = singles.tile([P, HC, S], F16)  # [p, hb, u] = enc[idx[u], hb*128+p]
    for (u0, ulen) in uchunks:
        src = enc_u1 if u0 == 0 else enc_u2
        for hb in range(HC):
            pt = psum.tile([P, P], F32, tag="tr")
            nc.tensor.transpose(pt[:, :ulen], src[:ulen, hb * P:(hb + 1) * P],
                                ident[:ulen, :ulen])
            (nc.vector.tensor_copy(out=enc_T[:, hb, u0:u0 + ulen], in_=pt[:, :ulen])
             if hb % 2 == 0 else
             nc.scalar.copy(out=enc_T[:, hb, u0:u0 + ulen], in_=pt[:, :ulen]))
    ew_T = singles.tile([P, HC, S], F16)  # [p, kc, u] = ew[idx[u], kc*128+p]
    for kc in range(HC):
        pew = psum.tile([P, S], F32, tag="proj")
        for hb in range(HC):
            nc.tensor.matmul(pew, lhsT=w1_16[:, hb, kc * P:(kc + 1) * P],
                             rhs=enc_T[:, hb, :], start=(hb == 0), stop=(hb == HC - 1))
        (nc.vector.tensor_copy(out=ew_T[:, kc, :], in_=pew) if kc % 2 == 0
         else nc.scalar.copy(out=ew_T[:, kc, :], in_=pew))

    # ---------------- derived constants ----------------
    # Built on the otherwise-idle GPSIMD engine and emitted after the
    # projections so they never head-of-line block the in-order DVE stream
    # (they are only needed once the first V-matmuls run, ~20us later).
    # V32[:, c, j, :] is a [128, 32] one-hot-column stationary operand:
    # column j holds V chunk c, all other columns zero.  A matmul with it
    # accumulates V_c . Y into score row (strip_base + j) while adding zero
    # to the other 31 rows of the strip (PSUM out must be 32-row aligned).
    v32 = singles.tile([P, HC, 32, 32], F16)
    _v32ms = nc.gpsimd.memset(v32, 0.0)
    # keep the dep-free memset from being hoisted in front of the gather
    # dispatches on the in-order gpsimd engine
    bass._add_dep_helper(_v32ms.ins, _gather_last.ins, sync=False,
                         reason="order v32 memset after enc gather dispatch")
    for c in range(HC):
        v32c = v32[:, c]
        diag = bass.AP(tensor=v32c.tensor, offset=v32c.offset,
                       ap=[v32c.ap[0], [33, 32]])
        nc.gpsimd.tensor_scalar_add(out=diag, in0=diag, scalar1=v_sb32[:, c:c + 1])

    # scatter matrix G[u, s] = 1 iff idx[u] == s (padding rows hit s=0 but
    # carry exactly-0 attention, so they contribute nothing)
    idx_f = singles.tile([P, 2], F32)
    for (u0, ulen) in uchunks:
        ci = u0 // P
        nc.gpsimd.tensor_copy(out=idx_f[:ulen, ci:ci + 1],
                              in_=idx_sb[:ulen, ci:ci + 1])
    G = singles.tile([P, 2, TE], F32)
    for (u0, ulen) in uchunks:
        nc.gpsimd.tensor_scalar(out=G[:ulen, u0 // P, :], in0=iota_f[:ulen, :],
                                scalar1=idx_f[:ulen, u0 // P:u0 // P + 1], scalar2=None,
                                op0=mybir.AluOpType.is_equal)

    # ---------------- score accumulation ----------------
    score_ps = score_pool.tile([P, TE], F32)  # [t, u]
    # mask bias broadcast into every row t: ones[1,128].T @ mask_bias[1,256]
    nc.tensor.matmul(score_ps, lhsT=ones_row, rhs=mask_bias, start=True, stop=False,
                     skip_group_check=True)

    # Group-size schedule: small first groups let the ACT stream start as
    # soon as possible (less exposed X-build latency); small last groups
    # shorten the final tanh->V-matmul burst before the softmax.
    group_sizes = [2, 6] + [TG] * 14 + [4, 2, 2]
    assert sum(group_sizes) == TD
    t0g = 0
    for gi, tg in enumerate(group_sizes):
        X = xpool.tile([P, TG * HC * S], F16)
        Y = ypool.tile([P, TG * HC * S], F16)
        for tl in range(tg):
            t = t0g + tl
            for c in range(HC):
                j = tl * HC + c
                # ~1/16 of the adds go to ScalarE (as Identity+bias
                # activations) to keep DVE under the ACT tanh rate
                if j % 16 == 15:
                    nc.scalar.add(out=X[:, j * S:(j + 1) * S],
                                  in_=ew_T[:, c, :], add=dw_T[:, c, t:t + 1])
                else:
                    nc.vector.tensor_scalar(out=X[:, j * S:(j + 1) * S],
                                            in0=ew_T[:, c, :],
                                            scalar1=dw_T[:, c, t:t + 1],
                                            scalar2=None, op0=add)
        nc.scalar.activation(out=Y[:, :tg * HC * S], in_=X[:, :tg * HC * S],
                             func=mybir.ActivationFunctionType.Tanh)
        for tl in range(tg):
            t = t0g + tl
            strip = (t // 32) * 32
            jj = t % 32
            for c in range(HC):
                j = tl * HC + c
                last = (t == TD - 1) and (c == HC - 1)
                nc.tensor.matmul(score_ps[strip:strip + 32, 0:S],
                                 lhsT=v32[:, c, jj, :],
                                 rhs=Y[:, j * S:(j + 1) * S], start=False, stop=last,
                                 skip_group_check=True, tile_position=(0, strip))
        t0g += tg

    # ---------------- softmax over u (no max-subtraction needed:
    # |score| <= sum|V| ~ 16, exp fits easily in fp32) ----------------
    p_sb = singles.tile([P, TE], F32)
    nc.scalar.activation(out=p_sb, in_=score_ps, func=mybir.ActivationFunctionType.Exp)
    den = singles.tile([P, 1], F32)
    nc.vector.tensor_reduce(out=den, in_=p_sb, axis=mybir.AxisListType.X, op=add)
    rec = singles.tile([P, 1], F32)
    nc.vector.reciprocal(out=rec, in_=den)
    attn_sb = singles.tile([P, TE], F32)  # permuted columns
    nc.vector.tensor_scalar(out=attn_sb, in0=p_sb, scalar1=rec[:, 0:1], scalar2=None,
                            op0=mult)

    # ---------------- context + attention scatter ----------------
    # attn_T [u, t] feeds both the context matmul (rhs = gathered enc) and
    # the scatter matmul (rhs = one-hot G) that un-permutes the columns.
    attn_T = singles.tile([P, 2, P], F32)
    for (u0, ulen) in uchunks:
        pt = psum.tile([P, P], F32, tag="tr")
        nc.tensor.transpose(pt[:ulen, :], attn_sb[:, u0:u0 + ulen], ident)
        nc.vector.tensor_copy(out=attn_T[:ulen, u0 // P, :], in_=pt[:ulen, :])
    ctx_ps = ctx_psum_pool.tile([P, H], F32)
    for ci, (u0, ulen) in enumerate(uchunks):
        src = enc_u1 if u0 == 0 else enc_u2
        nc.tensor.matmul(ctx_ps, lhsT=attn_T[:ulen, u0 // P, :], rhs=src[:ulen, :],
                         start=(ci == 0), stop=(ci == len(uchunks) - 1))
    ctx_sb = singles.tile([P, H], F32)
    nc.vector.tensor_copy(out=ctx_sb, in_=ctx_ps)
    nc.sync.dma_start(out=ctx_out, in_=ctx_sb)
    attn_ps = psum.tile([P, TE], F32, tag="proj")
    for ci, (u0, ulen) in enumerate(uchunks):
        nc.tensor.matmul(attn_ps, lhsT=attn_T[:ulen, u0 // P, :],
                         rhs=G[:ulen, u0 // P, :],
                         start=(ci == 0), stop=(ci == len(uchunks) - 1))
    attn_full = singles.tile([P, TE], F32)
    nc.vector.tensor_copy(out=attn_full, in_=attn_ps)
    nc.sync.dma_start(out=attn_out, in_=attn_full)


def build(S):
    nc = bacc.Bacc("TRN2", target_bir_lowering=False, debug=False, num_devices=B)
    enc = nc.dram_tensor("enc", (TE, H), F32, kind="ExternalInput").ap()
    dec = nc.dram_tensor("dec", (TD, H), F32, kind="ExternalInput").ap()
    idx = nc.dram_tensor("idx", (S,), I32, kind="ExternalInput").ap()
    sprime = nc.dram_tensor("sprime", (1,), F32, kind="ExternalInput").ap()
    w1 = nc.dram_tensor("w1", (H, H), F32, kind="ExternalInput").ap()
    w2 = nc.dram_tensor("w2", (H, H), F32, kind="ExternalInput").ap()
    v = nc.dram_tensor("v", (H, 1), F32, kind="ExternalInput").ap()
    ctx_out = nc.dram_tensor("ctx_out", (TD, H), F32, kind="ExternalOutput").ap()
    attn_out = nc.dram_tensor("attn_out", (TD, TE), F32, kind="ExternalOutput").ap()
    with tile.TileContext(nc) as tc:
        _attention_kernel(tc, S, enc, dec, idx, sprime, w1, w2, v, ctx_out, attn_out)
    nc.compile()
    return nc


_NC_CACHE = {}


def _get_nc(S):
    if S not in _NC_CACHE:
        _NC_CACHE[S] = build(S)
    return _NC_CACHE[S]


def make_in_maps(encoder_outputs, decoder_outputs, encoder_mask, W1, W2, V):
    enc = np.ascontiguousarray(np.asarray(encoder_outputs, dtype=np.float32))
    dec = np.ascontiguousarray(np.asarray(decoder_outputs, dtype=np.float32))
    msk = np.asarray(encoder_mask).astype(bool)
    w1 = np.ascontiguousarray(np.asarray(W1, dtype=np.float32))
    w2 = np.ascontiguousarray(np.asarray(W2, dtype=np.float32))
    v = np.ascontiguousarray(np.asarray(V, dtype=np.float32))
    us = [np.nonzero(msk[b])[0].astype(np.int32) for b in range(B)]
    max_sp = max(max((len(u) for u in us), default=1), 1)
    S = min(TE, ((max_sp + 15) // 16) * 16)
    in_maps = []
    for b in range(B):
        idx = np.zeros(S, np.int32)
        idx[:len(us[b])] = us[b]
        in_maps.append({
            "enc": enc[b], "dec": dec[b], "idx": idx,
            "sprime": np.array([len(us[b])], np.float32),
            "w1": w1, "w2": w2, "v": v,
        })
    return in_maps, S


def kernel(encoder_outputs, decoder_outputs, encoder_mask, W1, W2, V, **run_kwargs):
    in_maps, S = make_in_maps(encoder_outputs, decoder_outputs, encoder_mask, W1, W2, V)
    nc = _get_nc(S)
    res = bass_utils.run_bass_kernel_spmd(nc, in_maps, core_ids=list(range(B)),
                                          **run_kwargs)
    ctx = np.stack([res.results[b]["ctx_out"] for b in range(B)])
    attn = np.stack([res.results[b]["attn_out"] for b in range(B)])
    return ctx, attn


# revision 38
# speedup vs baseline: 1.0219x; 1.0108x over previous
"""Bahdanau additive attention kernel for Trainium2 (Bass/Tile).

Shapes (hardcoded from the problem spec):
  encoder_outputs [8, 256, 512] f32, decoder_outputs [8, 128, 512] f32,
  encoder_mask    [8, 256] bool,  W1/W2 [512, 512] f32,  V [512, 1] f32.

Sharding: data-parallel over batch B=8 across the 8 NeuronCores; the
W1/W2/V weights are replicated.  Each core computes one batch element:
  ew = enc @ W1; dw = dec @ W2
  score[t,s] = sum_h V[h] * tanh(ew[s,h] + dw[t,h]) + mask_bias[s]
  attn = softmax_s(score); ctx = attn @ enc

Key algorithmic point: masked encoder positions get attention weight
EXACTLY 0.0 in the reference (exp(score - 1e9 - max) underflows), so the
expensive tanh volume only needs the unmasked positions.  The kernel
gathers the ~50% unmasked encoder rows (indices prepared host-side as
sharding metadata, gather itself via on-device indirect DMA), runs the
whole pipeline on S_PAD <= 256 compacted positions, and scatters the
attention row back to the full 256 columns with a one-hot matmul.

Engine split per core (the Td*S_PAD*H element inner part):
  - DVE + GPSIMD: build X[h, (t,c,u)] = ew_T + dw_T[:, t] via
    per-partition-scalar adds (2x DVE mode; 1/8 of tiles on GPSIMD)
  - ACT: tanh on [128, TG*4*S_PAD] tiles (amortizes the per-instruction
    SBUF access bubble; ACT is the throughput floor of this problem)
  - PE : score accumulation via one-hot-padded V matmuls (M=32 strips),
    mask add as a K=1 broadcast matmul, projections/transposes/context,
    and the final scatter matmul
"""

from contextlib import ExitStack

import numpy as np

import concourse.bass as bass
import concourse.tile as tile
from concourse import bacc, bass_utils, mybir
from concourse._compat import with_exitstack
from concourse.masks import make_identity

B, TD, TE, H = 8, 128, 256, 512
P = 128
HC = H // P  # 4 h-chunks
TG = 8  # t's per ACT group
F32 = mybir.dt.float32
F16 = mybir.dt.float16
I32 = mybir.dt.int32


@with_exitstack
def _attention_kernel(ctx: ExitStack, tc: tile.TileContext, S, enc, dec, idx, sprime,
                      w1, w2, v, ctx_out, attn_out):
    nc = tc.nc
    add = mybir.AluOpType.add
    mult = mybir.AluOpType.mult

    # u-chunks of the compacted encoder axis (partition-dim blocks)
    uchunks = [(0, min(S, P))] + ([(P, S - P)] if S > P else [])

    singles = ctx.enter_context(tc.tile_pool(name="singles", bufs=1))
    xpool = ctx.enter_context(tc.tile_pool(name="xpool", bufs=3))
    ypool = ctx.enter_context(tc.tile_pool(name="ypool", bufs=3))
    psum = ctx.enter_context(tc.tile_pool(name="psum", bufs=3, space="PSUM"))
    score_pool = ctx.enter_context(tc.tile_pool(name="score", bufs=1, space="PSUM"))
    ctx_psum_pool = ctx.enter_context(tc.tile_pool(name="ctxp", bufs=1, space="PSUM"))

    # ---------------- input-independent constants first ----------------
    # (make_identity ends with a GPSIMD drain that would otherwise serialize
    # behind any DMA already queued on the gpsimd engine)
    ident = singles.tile([P, P], F32)
    make_identity(nc, ident)
    iota_i = singles.tile([P, TE], I32)  # 0..255 in every partition row
    nc.gpsimd.iota(iota_i, pattern=[[1, TE]], base=0, channel_multiplier=0)
    iota_f = singles.tile([P, TE], F32)
    nc.vector.tensor_copy(out=iota_f, in_=iota_i)
    ones_row = singles.tile([1, P], F16)
    nc.vector.memset(ones_row, 1.0)
    # PE warmup during the DMA wait: dummy matmuls keep the PE busy past the
    # HAM activity window so the transposes/projections run at 2.4 GHz.
    warm_sb = singles.tile([P, P], F16)
    nc.vector.memset(warm_sb, 0.0)
    warm_ps = psum.tile([P, P], F32, tag="tr")
    for _ in range(14):
        nc.tensor.matmul(warm_ps, lhsT=warm_sb, rhs=warm_sb, start=True, stop=True)

    # ---------------- loads ----------------
    # Each DMA queue sustains only ~95 GB/s, so the ~2.3MB of inputs are
    # split across the three dispatch engines (sync/scalar/gpsimd), ordered
    # by when the consumer needs them: idx/dec -> enc gather/W2 -> W1.
    idx_sb = singles.tile([P, 2], I32)  # column ci holds idx[ci*128 + p]
    idx_r = idx.rearrange("(u o) -> u o", o=1)
    for (u0, ulen) in uchunks:
        nc.scalar.dma_start(out=idx_sb[:ulen, u0 // P:u0 // P + 1],
                            in_=idx_r[u0:u0 + ulen])
    sprime_sb = singles.tile([1, 1], F32)
    nc.scalar.dma_start(out=sprime_sb, in_=sprime.rearrange("(u o) -> u o", o=1))
    v_sb32 = singles.tile([P, HC], F32)  # [p, c] = V[c*128+p, 0]
    nc.scalar.dma_start(out=v_sb32, in_=v.rearrange("(c p) o -> p (c o)", p=P))
    dec_sb = singles.tile([P, H], F32)
    nc.sync.dma_start(out=dec_sb, in_=dec)
    # gathered encoder rows: enc_u[j] = enc[idx[j]]
    enc_u1 = singles.tile([P, H], F32)
    _gather_last = nc.gpsimd.indirect_dma_start(
        out=enc_u1[:uchunks[0][1], :], out_offset=None, in_=enc[:],
        in_offset=bass.IndirectOffsetOnAxis(ap=idx_sb[:uchunks[0][1], 0:1], axis=0))
    if len(uchunks) > 1:
        enc_u2 = singles.tile([P, H], F32)
        _gather_last = nc.gpsimd.indirect_dma_start(
            out=enc_u2[:uchunks[1][1], :], out_offset=None, in_=enc[:],
            in_offset=bass.IndirectOffsetOnAxis(
                ap=idx_sb[:uchunks[1][1], 1:2], axis=0))
    w1_sb = singles.tile([P, HC, H], F32)  # [p, hb, k] = W1[hb*128+p, k]
    w1_r = w1.rearrange("(hb p) k -> p hb k", p=P)
    nc.sync.dma_start(out=w1_sb[:, 0:2], in_=w1_r[:, 0:2])
    nc.scalar.dma_start(out=w1_sb[:, 2:4], in_=w1_r[:, 2:4])
    w2_sb = singles.tile([P, HC, H], F32)
    w2_r = w2.rearrange("(hb p) k -> p hb k", p=P)
    nc.scalar.dma_start(out=w2_sb[:, 0:2], in_=w2_r[:, 0:2])
    nc.sync.dma_start(out=w2_sb[:, 2:4], in_=w2_r[:, 2:4])
    # fp16 copies of the weights: fp32 matmuls lower to two PE passes, so
    # halving the projection dtype halves the critical-path PE work.  The
    # casts run on the otherwise-idle prologue DVE, one per DMA half.
    w1_16 = singles.tile([P, HC, H], F16)
    w2_16 = singles.tile([P, HC, H], F16)
    for hb in range(HC):
        nc.scalar.copy(out=w1_16[:, hb], in_=w1_sb[:, hb])
    for hb in range(HC):
        nc.scalar.copy(out=w2_16[:, hb], in_=w2_sb[:, hb])

    # permuted-column mask bias row: 0 for u < S', -60000 beyond (-1e9 like
    # the reference would overflow fp16; any bias <= -1e3 gives
    # exp(score + bias) == 0.0 exactly in fp32, matching the reference).
    # Built on DVE: the mask matmul is the PSUM start=True writer that every
    # V-matmul waits on, so it must not sit behind slow GPSIMD ops.
    mask_bias = singles.tile([1, TE], F16)
    nc.vector.tensor_scalar(out=mask_bias, in0=iota_f[0:1, :],
                            scalar1=sprime_sb[0:1, 0:1], scalar2=-6e4,
                            op0=mybir.AluOpType.is_ge, op1=mult)

    # ---------------- transposes + projections ----------------
    # dec chain first: dec_T [h, t] then dw_T [k, t] (the X-build scalars),
    # then the enc chain: enc_T [h, u] and ew_T [k, u].
    dec_T = singles.tile([P, HC, TD], F16)
    for hb in range(HC):
        pt = psum.tile([P, P], F32, tag="tr")
        nc.tensor.transpose(pt, dec_sb[:, hb * P:(hb + 1) * P], ident)
        nc.vector.tensor_copy(out=dec_T[:, hb, :], in_=pt)
    enc_T = singles.tile([P, HC, S], F16)  # [p, hb, u] = enc[idx[u], hb*128+p]
    for (u0, ulen) in uchunks:
        esrc = enc_u1 if u0 == 0 else enc_u2
        for hb in range(HC):
            pt = psum.tile([P, P], F32, tag="tr")
            nc.tensor.transpose(pt[:, :ulen], esrc[:ulen, hb * P:(hb + 1) * P],
                                ident[:ulen, :ulen])
            nc.vector.tensor_copy(out=enc_T[:, hb, u0:u0 + ulen], in_=pt[:, :ulen])
    ew_T = singles.tile([P, HC, S], F16)  # [p, kc, u] = ew[idx[u], kc*128+p]
    for kc in range(HC):
        pew = psum.tile([P, S], F32, tag="proj")
        for hb in range(HC):
            nc.tensor.matmul(pew, lhsT=w1_16[:, hb, kc * P:(kc + 1) * P],
                             rhs=enc_T[:, hb, :], start=(hb == 0), stop=(hb == HC - 1))
        (nc.vector.tensor_copy(out=ew_T[:, kc, :], in_=pew) if kc % 2 == 0
         else nc.scalar.copy(out=ew_T[:, kc, :], in_=pew))
    dw_T = singles.tile([P, HC, TD], F32)
    for kc in range(HC):
        pdw = psum.tile([P, TD], F32, tag="proj")
        for hb in range(HC):
            nc.tensor.matmul(pdw, lhsT=w2_16[:, hb, kc * P:(kc + 1) * P],
                             rhs=dec_T[:, hb, :], start=(hb == 0), stop=(hb == HC - 1))
        (nc.vector.tensor_copy(out=dw_T[:, kc, :], in_=pdw) if kc % 2 == 0
         else nc.scalar.copy(out=dw_T[:, kc, :], in_=pdw))

    # ---------------- derived constants ----------------
    # Built on the otherwise-idle GPSIMD engine and emitted after the
    # projections so they never head-of-line block the in-order DVE stream
    # (they are only needed once the first V-matmuls run, ~20us later).
    # V32[:, c, j, :] is a [128, 32] one-hot-column stationary operand:
    # column j holds V chunk c, all other columns zero.  A matmul with it
    # accumulates V_c . Y into score row (strip_base + j) while adding zero
    # to the other 31 rows of the strip (PSUM out must be 32-row aligned).
    v32 = singles.tile([P, HC, 32, 32], F16)
    _v32ms = nc.gpsimd.memset(v32, 0.0)
    # keep the dep-free memset from being hoisted in front of the gather
    # dispatches on the in-order gpsimd engine
    bass._add_dep_helper(_v32ms.ins, _gather_last.ins, sync=False,
                         reason="order v32 memset after enc gather dispatch")
    for c in range(HC):
        v32c = v32[:, c]
        diag = bass.AP(tensor=v32c.tensor, offset=v32c.offset,
                       ap=[v32c.ap[0], [33, 32]])
        nc.gpsimd.tensor_scalar_add(out=diag, in0=diag, scalar1=v_sb32[:, c:c + 1])

    # scatter matrix G[u, s] = 1 iff idx[u] == s (padding rows hit s=0 but
    # carry exactly-0 attention, so they contribute nothing)
    idx_f = singles.tile([P, 2], F32)
    for (u0, ulen) in uchunks:
        ci = u0 // P
        nc.gpsimd.tensor_copy(out=idx_f[:ulen, ci:ci + 1],
                              in_=idx_sb[:ulen, ci:ci + 1])
    G = singles.tile([P, 2, TE], F32)
    for (u0, ulen) in uchunks:
        nc.gpsimd.tensor_scalar(out=G[:ulen, u0 // P, :], in0=iota_f[:ulen, :],
                                scalar1=idx_f[:ulen, u0 // P:u0 // P + 1], scalar2=None,
                                op0=mybir.AluOpType.is_equal)

    # ---------------- score accumulation ----------------
    score_ps = score_pool.tile([P, TE], F32)  # [t, u]
    # mask bias broadcast into every row t: ones[1,128].T @ mask_bias[1,256]
    nc.tensor.matmul(score_ps, lhsT=ones_row, rhs=mask_bias, start=True, stop=False,
                     skip_group_check=True)

    # Group-size schedule: small first groups let the ACT stream start as
    # soon as possible (less exposed X-build latency); small last groups
    # shorten the final tanh->V-matmul burst before the softmax.
    group_sizes = [2, 6] + [TG] * 14 + [4, 2, 2]
    assert sum(group_sizes) == TD
    t0g = 0
    for gi, tg in enumerate(group_sizes):
        X = xpool.tile([P, TG * HC * S], F16)
        Y = ypool.tile([P, TG * HC * S], F16)
        for tl in range(tg):
            t = t0g + tl
            for c in range(HC):
                j = tl * HC + c
                # ~1/16 of the adds go to ScalarE (as Identity+bias
                # activations) to keep DVE under the ACT tanh rate
                if j % 16 == 15:
                    nc.scalar.add(out=X[:, j * S:(j + 1) * S],
                                  in_=ew_T[:, c, :], add=dw_T[:, c, t:t + 1])
                else:
                    nc.vector.tensor_scalar(out=X[:, j * S:(j + 1) * S],
                                            in0=ew_T[:, c, :],
                                            scalar1=dw_T[:, c, t:t + 1],
                                            scalar2=None, op0=add)
        nc.scalar.activation(out=Y[:, :tg * HC * S], in_=X[:, :tg * HC * S],
                             func=mybir.ActivationFunctionType.Tanh)
        for tl in range(tg):
            t = t0g + tl
            strip = (t // 32) * 32
            jj = t % 32
            for c in range(HC):
                j = tl * HC + c
                last = (t == TD - 1) and (c == HC - 1)
                nc.tensor.matmul(score_ps[strip:strip + 32, 0:S],
                                 lhsT=v32[:, c, jj, :],
                                 rhs=Y[:, j * S:(j + 1) * S], start=False, stop=last,
                                 skip_group_check=True, tile_position=(0, strip))
        t0g += tg

    # ---------------- softmax over u (no max-subtraction needed:
    # |score| <= sum|V| ~ 16, exp fits easily in fp32) ----------------
    p_sb = singles.tile([P, TE], F32)
    nc.scalar.activation(out=p_sb, in_=score_ps, func=mybir.ActivationFunctionType.Exp)
    den = singles.tile([P, 1], F32)
    nc.vector.tensor_reduce(out=den, in_=p_sb, axis=mybir.AxisListType.X, op=add)
    rec = singles.tile([P, 1], F32)
    nc.vector.reciprocal(out=rec, in_=den)
    attn_sb = singles.tile([P, TE], F32)  # permuted columns
    nc.vector.tensor_scalar(out=attn_sb, in0=p_sb, scalar1=rec[:, 0:1], scalar2=None,
                            op0=mult)

    # ---------------- context + attention scatter ----------------
    # attn_T [u, t] feeds both the context matmul (rhs = gathered enc) and
    # the scatter matmul (rhs = one-hot G) that un-permutes the columns.
    attn_T = singles.tile([P, 2, P], F32)
    for (u0, ulen) in uchunks:
        pt = psum.tile([P, P], F32, tag="tr")
        nc.tensor.transpose(pt[:ulen, :], attn_sb[:, u0:u0 + ulen], ident)
        nc.vector.tensor_copy(out=attn_T[:ulen, u0 // P, :], in_=pt[:ulen, :])
    ctx_ps = ctx_psum_pool.tile([P, H], F32)
    for ci, (u0, ulen) in enumerate(uchunks):
        src = enc_u1 if u0 == 0 else enc_u2
        nc.tensor.matmul(ctx_ps, lhsT=attn_T[:ulen, u0 // P, :], rhs=src[:ulen, :],
                         start=(ci == 0), stop=(ci == len(uchunks) - 1))
    ctx_sb = singles.tile([P, H], F32)
    nc.vector.tensor_copy(out=ctx_sb, in_=ctx_ps)
    nc.sync.dma_start(out=ctx_out, in_=ctx_sb)
    attn_ps = psum.tile([P, TE], F32, tag="proj")
    for ci, (u0, ulen) in enumerate(uchunks):
        nc.tensor.matmul(attn_ps, lhsT=attn_T[:ulen, u0 // P, :],
                         rhs=G[:ulen, u0 // P, :],
                         start=(ci == 0), stop=(ci == len(uchunks) - 1))
    attn_full = singles.tile([P, TE], F32)
    nc.vector.tensor_copy(out=attn_full, in_=attn_ps)
    nc.sync.dma_start(out=attn_out, in_=attn_full)


def build(S):
    nc = bacc.Bacc("TRN2", target_bir_lowering=False, debug=False, num_devices=B)
    enc = nc.dram_tensor("enc", (TE, H), F32, kind="ExternalInput").ap()
    dec = nc.dram_tensor("dec", (TD, H), F32, kind="ExternalInput").ap()
    idx = nc.dram_tensor("idx", (S,), I32, kind="ExternalInput").ap()
    sprime = nc.dram_tensor("sprime", (1,), F32, kind="ExternalInput").ap()
    w1 = nc.dram_tensor("w1", (H, H), F32, kind="ExternalInput").ap()
    w2 = nc.dram_tensor("w2", (H, H), F32, kind="ExternalInput").ap()
    v = nc.dram_tensor("v", (H, 1), F32, kind="ExternalInput").ap()
    ctx_out = nc.dram_tensor("ctx_out", (TD, H), F32, kind="ExternalOutput").ap()
    attn_out = nc.dram_tensor("attn_out", (TD, TE), F32, kind="ExternalOutput").ap()
    with tile.TileContext(nc) as tc:
        _attention_kernel(tc, S, enc, dec, idx, sprime, w1, w2, v, ctx_out, attn_out)
    nc.compile()
    return nc


_NC_CACHE = {}


def _get_nc(S):
    if S not in _NC_CACHE:
        _NC_CACHE[S] = build(S)
    return _NC_CACHE[S]


def make_in_maps(encoder_outputs, decoder_outputs, encoder_mask, W1, W2, V):
    enc = np.ascontiguousarray(np.asarray(encoder_outputs, dtype=np.float32))
    dec = np.ascontiguousarray(np.asarray(decoder_outputs, dtype=np.float32))
    msk = np.asarray(encoder_mask).astype(bool)
    w1 = np.ascontiguousarray(np.asarray(W1, dtype=np.float32))
    w2 = np.ascontiguousarray(np.asarray(W2, dtype=np.float32))
    v = np.ascontiguousarray(np.asarray(V, dtype=np.float32))
    us = [np.nonzero(msk[b])[0].astype(np.int32) for b in range(B)]
    max_sp = max(max((len(u) for u in us), default=1), 1)
    S = min(TE, ((max_sp + 15) // 16) * 16)
    in_maps = []
    for b in range(B):
        idx = np.zeros(S, np.int32)
        idx[:len(us[b])] = us[b]
        in_maps.append({
            "enc": enc[b], "dec": dec[b], "idx": idx,
            "sprime": np.array([len(us[b])], np.float32),
            "w1": w1, "w2": w2, "v": v,
        })
    return in_maps, S


def kernel(encoder_outputs, decoder_outputs, encoder_mask, W1, W2, V, **run_kwargs):
    in_maps, S = make_in_maps(encoder_outputs, decoder_outputs, encoder_mask, W1, W2, V)
    nc = _get_nc(S)
    res = bass_utils.run_bass_kernel_spmd(nc, in_maps, core_ids=list(range(B)),
                                          **run_kwargs)
    ctx = np.stack([res.results[b]["ctx_out"] for b in range(B)])
    attn = np.stack([res.results[b]["attn_out"] for b in range(B)])
    return ctx, attn


# revision 39
# speedup vs baseline: 1.0419x; 1.0196x over previous
"""Bahdanau additive attention kernel for Trainium2 (Bass/Tile).

Shapes (hardcoded from the problem spec):
  encoder_outputs [8, 256, 512] f32, decoder_outputs [8, 128, 512] f32,
  encoder_mask    [8, 256] bool,  W1/W2 [512, 512] f32,  V [512, 1] f32.

Sharding: data-parallel over batch B=8 across the 8 NeuronCores; the
W1/W2/V weights are replicated.  Each core computes one batch element:
  ew = enc @ W1; dw = dec @ W2
  score[t,s] = sum_h V[h] * tanh(ew[s,h] + dw[t,h]) + mask_bias[s]
  attn = softmax_s(score); ctx = attn @ enc

Key algorithmic point: masked encoder positions get attention weight
EXACTLY 0.0 in the reference (exp(score - 1e9 - max) underflows), so the
expensive tanh volume only needs the unmasked positions.  The kernel
gathers the ~50% unmasked encoder rows (indices prepared host-side as
sharding metadata, gather itself via on-device indirect DMA), runs the
whole pipeline on S_PAD <= 256 compacted positions, and scatters the
attention row back to the full 256 columns with a one-hot matmul.

Engine split per core (the Td*S_PAD*H element inner part):
  - DVE + GPSIMD: build X[h, (t,c,u)] = ew_T + dw_T[:, t] via
    per-partition-scalar adds (2x DVE mode; 1/8 of tiles on GPSIMD)
  - ACT: tanh on [128, TG*4*S_PAD] tiles (amortizes the per-instruction
    SBUF access bubble; ACT is the throughput floor of this problem)
  - PE : score accumulation via one-hot-padded V matmuls (M=32 strips),
    mask add as a K=1 broadcast matmul, projections/transposes/context,
    and the final scatter matmul
"""

from contextlib import ExitStack

import numpy as np

import concourse.bass as bass
import concourse.tile as tile
from concourse import bacc, bass_utils, mybir
from concourse._compat import with_exitstack
from concourse.masks import make_identity

B, TD, TE, H = 8, 128, 256, 512
P = 128
HC = H // P  # 4 h-chunks
TG = 8  # t's per ACT group
F32 = mybir.dt.float32
F16 = mybir.dt.float16
I32 = mybir.dt.int32


@with_exitstack
def _attention_kernel(ctx: ExitStack, tc: tile.TileContext, S, enc, dec, idx, sprime,
                      w1, w2, v, ctx_out, attn_out):
    nc = tc.nc
    add = mybir.AluOpType.add
    mult = mybir.AluOpType.mult

    # u-chunks of the compacted encoder axis (partition-dim blocks)
    uchunks = [(0, min(S, P))] + ([(P, S - P)] if S > P else [])

    singles = ctx.enter_context(tc.tile_pool(name="singles", bufs=1))
    xpool = ctx.enter_context(tc.tile_pool(name="xpool", bufs=3))
    ypool = ctx.enter_context(tc.tile_pool(name="ypool", bufs=3))
    psum = ctx.enter_context(tc.tile_pool(name="psum", bufs=3, space="PSUM"))
    score_pool = ctx.enter_context(tc.tile_pool(name="score", bufs=1, space="PSUM"))
    ctx_psum_pool = ctx.enter_context(tc.tile_pool(name="ctxp", bufs=1, space="PSUM"))

    # ---------------- input-independent constants first ----------------
    # (make_identity ends with a GPSIMD drain that would otherwise serialize
    # behind any DMA already queued on the gpsimd engine)
    ident = singles.tile([P, P], F32)
    make_identity(nc, ident)
    iota_i = singles.tile([P, TE], I32)  # 0..255 in every partition row
    nc.gpsimd.iota(iota_i, pattern=[[1, TE]], base=0, channel_multiplier=0)
    iota_f = singles.tile([P, TE], F32)
    nc.vector.tensor_copy(out=iota_f, in_=iota_i)
    ones_row = singles.tile([1, P], F16)
    nc.vector.memset(ones_row, 1.0)
    # PE warmup during the DMA wait: dummy matmuls keep the PE busy past the
    # HAM activity window so the transposes/projections run at 2.4 GHz.
    warm_sb = singles.tile([P, P], F16)
    nc.vector.memset(warm_sb, 0.0)
    warm_ps = psum.tile([P, P], F32, tag="tr")
    for _ in range(14):
        nc.tensor.matmul(warm_ps, lhsT=warm_sb, rhs=warm_sb, start=True, stop=True)

    # ---------------- loads ----------------
    # Each DMA queue sustains only ~95 GB/s, so the ~2.3MB of inputs are
    # split across the three dispatch engines (sync/scalar/gpsimd), ordered
    # by when the consumer needs them: idx/dec -> enc gather/W2 -> W1.
    idx_sb = singles.tile([P, 2], I32)  # column ci holds idx[ci*128 + p]
    idx_r = idx.rearrange("(u o) -> u o", o=1)
    for (u0, ulen) in uchunks:
        nc.scalar.dma_start(out=idx_sb[:ulen, u0 // P:u0 // P + 1],
                            in_=idx_r[u0:u0 + ulen])
    sprime_sb = singles.tile([1, 1], F32)
    nc.scalar.dma_start(out=sprime_sb, in_=sprime.rearrange("(u o) -> u o", o=1))
    v_sb32 = singles.tile([P, HC], F32)  # [p, c] = V[c*128+p, 0]
    nc.scalar.dma_start(out=v_sb32, in_=v.rearrange("(c p) o -> p (c o)", p=P))
    dec_sb = singles.tile([P, H], F32)
    nc.sync.dma_start(out=dec_sb, in_=dec)
    # gathered encoder rows: enc_u[j] = enc[idx[j]]
    enc_u1 = singles.tile([P, H], F32)
    _gather_last = nc.gpsimd.indirect_dma_start(
        out=enc_u1[:uchunks[0][1], :], out_offset=None, in_=enc[:],
        in_offset=bass.IndirectOffsetOnAxis(ap=idx_sb[:uchunks[0][1], 0:1], axis=0))
    if len(uchunks) > 1:
        enc_u2 = singles.tile([P, H], F32)
        _gather_last = nc.gpsimd.indirect_dma_start(
            out=enc_u2[:uchunks[1][1], :], out_offset=None, in_=enc[:],
            in_offset=bass.IndirectOffsetOnAxis(
                ap=idx_sb[:uchunks[1][1], 1:2], axis=0))
    w1_sb = singles.tile([P, HC, H], F32)  # [p, hb, k] = W1[hb*128+p, k]
    w1_r = w1.rearrange("(hb p) k -> p hb k", p=P)
    nc.sync.dma_start(out=w1_sb[:, 0:2], in_=w1_r[:, 0:2])
    nc.scalar.dma_start(out=w1_sb[:, 2:4], in_=w1_r[:, 2:4])
    w2_sb = singles.tile([P, HC, H], F32)
    w2_r = w2.rearrange("(hb p) k -> p hb k", p=P)
    nc.scalar.dma_start(out=w2_sb[:, 0:2], in_=w2_r[:, 0:2])
    nc.sync.dma_start(out=w2_sb[:, 2:4], in_=w2_r[:, 2:4])
    # fp16 copies of the weights: fp32 matmuls lower to two PE passes, so
    # halving the projection dtype halves the critical-path PE work.  The
    # casts run on the otherwise-idle prologue DVE, one per DMA half.
    w1_16 = singles.tile([P, HC, H], F16)
    w2_16 = singles.tile([P, HC, H], F16)
    for hb in range(HC):
        nc.scalar.copy(out=w1_16[:, hb], in_=w1_sb[:, hb])
    for hb in range(HC):
        nc.scalar.copy(out=w2_16[:, hb], in_=w2_sb[:, hb])

    # permuted-column mask bias row: 0 for u < S', -60000 beyond (-1e9 like
    # the reference would overflow fp16; any bias <= -1e3 gives
    # exp(score + bias) == 0.0 exactly in fp32, matching the reference).
    # Built on DVE: the mask matmul is the PSUM start=True writer that every
    # V-matmul waits on, so it must not sit behind slow GPSIMD ops.
    mask_bias = singles.tile([1, TE], F16)
    nc.vector.tensor_scalar(out=mask_bias, in0=iota_f[0:1, :],
                            scalar1=sprime_sb[0:1, 0:1], scalar2=-6e4,
                            op0=mybir.AluOpType.is_ge, op1=mult)

    # ---------------- transposes + projections ----------------
    # dec chain first: dec_T [h, t] then dw_T [k, t] (the X-build scalars),
    # then the enc chain: enc_T [h, u] and ew_T [k, u].
    dec_T = singles.tile([P, HC, TD], F16)
    for hb in range(HC):
        pt = psum.tile([P, P], F32, tag="tr")
        nc.tensor.transpose(pt, dec_sb[:, hb * P:(hb + 1) * P], ident)
        nc.vector.tensor_copy(out=dec_T[:, hb, :], in_=pt)
    enc_T = singles.tile([P, HC, S], F16)  # [p, hb, u] = enc[idx[u], hb*128+p]
    for (u0, ulen) in uchunks:
        esrc = enc_u1 if u0 == 0 else enc_u2
        for hb in range(HC):
            pt = psum.tile([P, P], F32, tag="tr")
            nc.tensor.transpose(pt[:, :ulen], esrc[:ulen, hb * P:(hb + 1) * P],
                                ident[:ulen, :ulen])
            nc.vector.tensor_copy(out=enc_T[:, hb, u0:u0 + ulen], in_=pt[:, :ulen])
    ew_T = singles.tile([P, HC, S], F16)  # [p, kc, u] = ew[idx[u], kc*128+p]
    for kc in range(HC):
        pew = psum.tile([P, S], F32, tag="proj")
        for hb in range(HC):
            nc.tensor.matmul(pew, lhsT=w1_16[:, hb, kc * P:(kc + 1) * P],
                             rhs=enc_T[:, hb, :], start=(hb == 0), stop=(hb == HC - 1))
        (nc.vector.tensor_copy(out=ew_T[:, kc, :], in_=pew) if kc % 2 == 0
         else nc.scalar.copy(out=ew_T[:, kc, :], in_=pew))
    dw_T = singles.tile([P, HC, TD], F32)
    for kc in range(HC):
        pdw = psum.tile([P, TD], F32, tag="proj")
        for hb in range(HC):
            nc.tensor.matmul(pdw, lhsT=w2_16[:, hb, kc * P:(kc + 1) * P],
                             rhs=dec_T[:, hb, :], start=(hb == 0), stop=(hb == HC - 1))
        (nc.vector.tensor_copy(out=dw_T[:, kc, :], in_=pdw) if kc % 2 == 0
         else nc.scalar.copy(out=dw_T[:, kc, :], in_=pdw))

    # ---------------- derived constants ----------------
    # Built on the otherwise-idle GPSIMD engine and emitted after the
    # projections so they never head-of-line block the in-order DVE stream
    # (they are only needed once the first V-matmuls run, ~20us later).
    # V32[:, c, j, :] is a [128, 32] one-hot-column stationary operand:
    # column j holds V chunk c, all other columns zero.  A matmul with it
    # accumulates V_c . Y into score row (strip_base + j) while adding zero
    # to the other 31 rows of the strip (PSUM out must be 32-row aligned).
    v32 = singles.tile([P, HC, 32, 32], F16)
    _v32ms = nc.gpsimd.memset(v32, 0.0)
    # keep the dep-free memset from being hoisted in front of the gather
    # dispatches on the in-order gpsimd engine
    bass._add_dep_helper(_v32ms.ins, _gather_last.ins, sync=False,
                         reason="order v32 memset after enc gather dispatch")
    for c in range(HC):
        v32c = v32[:, c]
        diag = bass.AP(tensor=v32c.tensor, offset=v32c.offset,
                       ap=[v32c.ap[0], [33, 32]])
        nc.gpsimd.tensor_scalar_add(out=diag, in0=diag, scalar1=v_sb32[:, c:c + 1])

    # scatter matrix G[u, s] = 1 iff idx[u] == s (padding rows hit s=0 but
    # carry exactly-0 attention, so they contribute nothing)
    idx_f = singles.tile([P, 2], F32)
    for (u0, ulen) in uchunks:
        ci = u0 // P
        nc.gpsimd.tensor_copy(out=idx_f[:ulen, ci:ci + 1],
                              in_=idx_sb[:ulen, ci:ci + 1])
    G = singles.tile([P, 2, TE], F32)
    for (u0, ulen) in uchunks:
        nc.gpsimd.tensor_scalar(out=G[:ulen, u0 // P, :], in0=iota_f[:ulen, :],
                                scalar1=idx_f[:ulen, u0 // P:u0 // P + 1], scalar2=None,
                                op0=mybir.AluOpType.is_equal)

    # ---------------- score accumulation ----------------
    score_ps = score_pool.tile([P, TE], F32)  # [t, u]
    # mask bias broadcast into every row t: ones[1,128].T @ mask_bias[1,256]
    nc.tensor.matmul(score_ps, lhsT=ones_row, rhs=mask_bias, start=True, stop=False,
                     skip_group_check=True)

    # Group-size schedule: small first groups let the ACT stream start as
    # soon as possible (less exposed X-build latency); small last groups
    # shorten the final tanh->V-matmul burst before the softmax.
    group_sizes = [2, 6] + [TG] * 14 + [4, 2, 2]
    assert sum(group_sizes) == TD
    t0g = 0
    for gi, tg in enumerate(group_sizes):
        X = xpool.tile([P, TG * HC * S], F16)
        Y = ypool.tile([P, TG * HC * S], F16)
        # Steady-state groups compute their last t as four fused bias-tanh
        # activations on ScalarE (no X traffic, no DVE work): this balances
        # DVE's 2x-mode add rate against ACT's tanh rate.  The first two
        # groups stay pure-DVE so no extra ACT instructions sit in front of
        # the first big tanh (measured prologue regression otherwise).
        fused = (2 <= gi <= 15)
        n_dve_t = tg - 1 if fused else tg
        for tl in range(n_dve_t):
            t = t0g + tl
            for c in range(HC):
                j = tl * HC + c
                # ~1/16 of the adds go to ScalarE (as Identity+bias
                # activations) to keep DVE under the ACT tanh rate
                if not fused and j % 16 == 15:
                    nc.scalar.add(out=X[:, j * S:(j + 1) * S],
                                  in_=ew_T[:, c, :], add=dw_T[:, c, t:t + 1])
                else:
                    nc.vector.tensor_scalar(out=X[:, j * S:(j + 1) * S],
                                            in0=ew_T[:, c, :],
                                            scalar1=dw_T[:, c, t:t + 1],
                                            scalar2=None, op0=add)
        if fused:
            t = t0g + tg - 1
            for c in range(HC):
                j = (tg - 1) * HC + c
                nc.scalar.activation(out=Y[:, j * S:(j + 1) * S],
                                     in_=ew_T[:, c, :],
                                     func=mybir.ActivationFunctionType.Tanh,
                                     bias=dw_T[:, c, t:t + 1], scale=1.0)
        nc.scalar.activation(out=Y[:, :n_dve_t * HC * S],
                             in_=X[:, :n_dve_t * HC * S],
                             func=mybir.ActivationFunctionType.Tanh)
        for tl in range(tg):
            t = t0g + tl
            strip = (t // 32) * 32
            jj = t % 32
            for c in range(HC):
                j = tl * HC + c
                last = (t == TD - 1) and (c == HC - 1)
                nc.tensor.matmul(score_ps[strip:strip + 32, 0:S],
                                 lhsT=v32[:, c, jj, :],
                                 rhs=Y[:, j * S:(j + 1) * S], start=False, stop=last,
                                 skip_group_check=True, tile_position=(0, strip))
        t0g += tg

    # ---------------- softmax over u (no max-subtraction needed:
    # |score| <= sum|V| ~ 16, exp fits easily in fp32) ----------------
    p_sb = singles.tile([P, TE], F32)
    nc.scalar.activation(out=p_sb, in_=score_ps, func=mybir.ActivationFunctionType.Exp)
    den = singles.tile([P, 1], F32)
    nc.vector.tensor_reduce(out=den, in_=p_sb, axis=mybir.AxisListType.X, op=add)
    rec = singles.tile([P, 1], F32)
    nc.vector.reciprocal(out=rec, in_=den)
    attn_sb = singles.tile([P, TE], F32)  # permuted columns
    nc.vector.tensor_scalar(out=attn_sb, in0=p_sb, scalar1=rec[:, 0:1], scalar2=None,
                            op0=mult)

    # ---------------- context + attention scatter ----------------
    # attn_T [u, t] feeds both the context matmul (rhs = gathered enc) and
    # the scatter matmul (rhs = one-hot G) that un-permutes the columns.
    attn_T = singles.tile([P, 2, P], F32)
    for (u0, ulen) in uchunks:
        pt = psum.tile([P, P], F32, tag="tr")
        nc.tensor.transpose(pt[:ulen, :], attn_sb[:, u0:u0 + ulen], ident)
        nc.vector.tensor_copy(out=attn_T[:ulen, u0 // P, :], in_=pt[:ulen, :])
    ctx_ps = ctx_psum_pool.tile([P, H], F32)
    for ci, (u0, ulen) in enumerate(uchunks):
        src = enc_u1 if u0 == 0 else enc_u2
        nc.tensor.matmul(ctx_ps, lhsT=attn_T[:ulen, u0 // P, :], rhs=src[:ulen, :],
                         start=(ci == 0), stop=(ci == len(uchunks) - 1))
    ctx_sb = singles.tile([P, H], F32)
    nc.vector.tensor_copy(out=ctx_sb, in_=ctx_ps)
    nc.sync.dma_start(out=ctx_out, in_=ctx_sb)
    attn_ps = psum.tile([P, TE], F32, tag="proj")
    for ci, (u0, ulen) in enumerate(uchunks):
        nc.tensor.matmul(attn_ps, lhsT=attn_T[:ulen, u0 // P, :],
                         rhs=G[:ulen, u0 // P, :],
                         start=(ci == 0), stop=(ci == len(uchunks) - 1))
    attn_full = singles.tile([P, TE], F32)
    nc.vector.tensor_copy(out=attn_full, in_=attn_ps)
    nc.sync.dma_start(out=attn_out, in_=attn_full)


def build(S):
    nc = bacc.Bacc("TRN2", target_bir_lowering=False, debug=False, num_devices=B)
    enc = nc.dram_tensor("enc", (TE, H), F32, kind="ExternalInput").ap()
    dec = nc.dram_tensor("dec", (TD, H), F32, kind="ExternalInput").ap()
    idx = nc.dram_tensor("idx", (S,), I32, kind="ExternalInput").ap()
    sprime = nc.dram_tensor("sprime", (1,), F32, kind="ExternalInput").ap()
    w1 = nc.dram_tensor("w1", (H, H), F32, kind="ExternalInput").ap()
    w2 = nc.dram_tensor("w2", (H, H), F32, kind="ExternalInput").ap()
    v = nc.dram_tensor("v", (H, 1), F32, kind="ExternalInput").ap()
    ctx_out = nc.dram_tensor("ctx_out", (TD, H), F32, kind="ExternalOutput").ap()
    attn_out = nc.dram_tensor("attn_out", (TD, TE), F32, kind="ExternalOutput").ap()
    with tile.TileContext(nc) as tc:
        _attention_kernel(tc, S, enc, dec, idx, sprime, w1, w2, v, ctx_out, attn_out)
    nc.compile()
    return nc


_NC_CACHE = {}


def _get_nc(S):
    if S not in _NC_CACHE:
        _NC_CACHE[S] = build(S)
    return _NC_CACHE[S]


def make_in_maps(encoder_outputs, decoder_outputs, encoder_mask, W1, W2, V):
    enc = np.ascontiguousarray(np.asarray(encoder_outputs, dtype=np.float32))
    dec = np.ascontiguousarray(np.asarray(decoder_outputs, dtype=np.float32))
    msk = np.asarray(encoder_mask).astype(bool)
    w1 = np.ascontiguousarray(np.asarray(W1, dtype=np.float32))
    w2 = np.ascontiguousarray(np.asarray(W2, dtype=np.float32))
    v = np.ascontiguousarray(np.asarray(V, dtype=np.float32))
    us = [np.nonzero(msk[b])[0].astype(np.int32) for b in range(B)]
    max_sp = max(max((len(u) for u in us), default=1), 1)
    S = min(TE, ((max_sp + 15) // 16) * 16)
    in_maps = []
    for b in range(B):
        idx = np.zeros(S, np.int32)
        idx[:len(us[b])] = us[b]
        in_maps.append({
            "enc": enc[b], "dec": dec[b], "idx": idx,
            "sprime": np.array([len(us[b])], np.float32),
            "w1": w1, "w2": w2, "v": v,
        })
    return in_maps, S


def kernel(encoder_outputs, decoder_outputs, encoder_mask, W1, W2, V, **run_kwargs):
    in_maps, S = make_in_maps(encoder_outputs, decoder_outputs, encoder_mask, W1, W2, V)
    nc = _get_nc(S)
    res = bass_utils.run_bass_kernel_spmd(nc, in_maps, core_ids=list(range(B)),
                                          **run_kwargs)
    ctx = np.stack([res.results[b]["ctx_out"] for b in range(B)])
    attn = np.stack([res.results[b]["attn_out"] for b in range(B)])
    return ctx, attn


# revision 40
# speedup vs baseline: 1.0430x; 1.0010x over previous
"""Bahdanau additive attention kernel for Trainium2 (Bass/Tile).

Shapes (hardcoded from the problem spec):
  encoder_outputs [8, 256, 512] f32, decoder_outputs [8, 128, 512] f32,
  encoder_mask    [8, 256] bool,  W1/W2 [512, 512] f32,  V [512, 1] f32.

Sharding: data-parallel over batch B=8 across the 8 NeuronCores; the
W1/W2/V weights are replicated.  Each core computes one batch element:
  ew = enc @ W1; dw = dec @ W2
  score[t,s] = sum_h V[h] * tanh(ew[s,h] + dw[t,h]) + mask_bias[s]
  attn = softmax_s(score); ctx = attn @ enc

Key algorithmic point: masked encoder positions get attention weight
EXACTLY 0.0 in the reference (exp(score - 1e9 - max) underflows), so the
expensive tanh volume only needs the unmasked positions.  The kernel
gathers the ~50% unmasked encoder rows (indices prepared host-side as
sharding metadata, gather itself via on-device indirect DMA), runs the
whole pipeline on S_PAD <= 256 compacted positions, and scatters the
attention row back to the full 256 columns with a one-hot matmul.

Engine split per core (the Td*S_PAD*H element inner part):
  - DVE + GPSIMD: build X[h, (t,c,u)] = ew_T + dw_T[:, t] via
    per-partition-scalar adds (2x DVE mode; 1/8 of tiles on GPSIMD)
  - ACT: tanh on [128, TG*4*S_PAD] tiles (amortizes the per-instruction
    SBUF access bubble; ACT is the throughput floor of this problem)
  - PE : score accumulation via one-hot-padded V matmuls (M=32 strips),
    mask add as a K=1 broadcast matmul, projections/transposes/context,
    and the final scatter matmul
"""

from contextlib import ExitStack

import numpy as np

import concourse.bass as bass
import concourse.tile as tile
from concourse import bacc, bass_utils, mybir
from concourse._compat import with_exitstack
from concourse.masks import make_identity

B, TD, TE, H = 8, 128, 256, 512
P = 128
HC = H // P  # 4 h-chunks
TG = 8  # t's per ACT group
F32 = mybir.dt.float32
F16 = mybir.dt.float16
I32 = mybir.dt.int32


@with_exitstack
def _attention_kernel(ctx: ExitStack, tc: tile.TileContext, S, enc, dec, idx, sprime,
                      w1, w2, v, ctx_out, attn_out):
    nc = tc.nc
    add = mybir.AluOpType.add
    mult = mybir.AluOpType.mult

    # u-chunks of the compacted encoder axis (partition-dim blocks)
    uchunks = [(0, min(S, P))] + ([(P, S - P)] if S > P else [])

    singles = ctx.enter_context(tc.tile_pool(name="singles", bufs=1))
    xpool = ctx.enter_context(tc.tile_pool(name="xpool", bufs=3))
    ypool = ctx.enter_context(tc.tile_pool(name="ypool", bufs=3))
    psum = ctx.enter_context(tc.tile_pool(name="psum", bufs=3, space="PSUM"))
    score_pool = ctx.enter_context(tc.tile_pool(name="score", bufs=1, space="PSUM"))
    ctx_psum_pool = ctx.enter_context(tc.tile_pool(name="ctxp", bufs=1, space="PSUM"))

    # ---------------- input-independent constants first ----------------
    # (make_identity ends with a GPSIMD drain that would otherwise serialize
    # behind any DMA already queued on the gpsimd engine)
    ident = singles.tile([P, P], F32)
    make_identity(nc, ident)
    iota_i = singles.tile([P, TE], I32)  # 0..255 in every partition row
    nc.gpsimd.iota(iota_i, pattern=[[1, TE]], base=0, channel_multiplier=0)
    iota_f = singles.tile([P, TE], F32)
    nc.vector.tensor_copy(out=iota_f, in_=iota_i)
    ones_row = singles.tile([1, P], F16)
    nc.vector.memset(ones_row, 1.0)
    # PE warmup during the DMA wait: dummy matmuls keep the PE busy past the
    # HAM activity window so the transposes/projections run at 2.4 GHz.
    warm_sb = singles.tile([P, P], F16)
    nc.vector.memset(warm_sb, 0.0)
    warm_ps = psum.tile([P, P], F32, tag="tr")
    for _ in range(14):
        nc.tensor.matmul(warm_ps, lhsT=warm_sb, rhs=warm_sb, start=True, stop=True)

    # ---------------- loads ----------------
    # Each DMA queue sustains only ~95 GB/s, so the ~2.3MB of inputs are
    # split across the three dispatch engines (sync/scalar/gpsimd), ordered
    # by when the consumer needs them: idx/dec -> enc gather/W2 -> W1.
    idx_sb = singles.tile([P, 2], I32)  # column ci holds idx[ci*128 + p]
    idx_r = idx.rearrange("(u o) -> u o", o=1)
    for (u0, ulen) in uchunks:
        nc.scalar.dma_start(out=idx_sb[:ulen, u0 // P:u0 // P + 1],
                            in_=idx_r[u0:u0 + ulen])
    sprime_sb = singles.tile([1, 1], F32)
    nc.scalar.dma_start(out=sprime_sb, in_=sprime.rearrange("(u o) -> u o", o=1))
    v_sb32 = singles.tile([P, HC], F32)  # [p, c] = V[c*128+p, 0]
    nc.scalar.dma_start(out=v_sb32, in_=v.rearrange("(c p) o -> p (c o)", p=P))
    dec_sb = singles.tile([P, H], F32)
    nc.sync.dma_start(out=dec_sb, in_=dec)
    # gathered encoder rows: enc_u[j] = enc[idx[j]]
    enc_u1 = singles.tile([P, H], F32)
    _gather_last = nc.gpsimd.indirect_dma_start(
        out=enc_u1[:uchunks[0][1], :], out_offset=None, in_=enc[:],
        in_offset=bass.IndirectOffsetOnAxis(ap=idx_sb[:uchunks[0][1], 0:1], axis=0))
    if len(uchunks) > 1:
        enc_u2 = singles.tile([P, H], F32)
        _gather_last = nc.gpsimd.indirect_dma_start(
            out=enc_u2[:uchunks[1][1], :], out_offset=None, in_=enc[:],
            in_offset=bass.IndirectOffsetOnAxis(
                ap=idx_sb[:uchunks[1][1], 1:2], axis=0))
    w1_sb = singles.tile([P, HC, H], F32)  # [p, hb, k] = W1[hb*128+p, k]
    w1_r = w1.rearrange("(hb p) k -> p hb k", p=P)
    nc.sync.dma_start(out=w1_sb[:, 0:2], in_=w1_r[:, 0:2])
    nc.scalar.dma_start(out=w1_sb[:, 2:4], in_=w1_r[:, 2:4])
    w2_sb = singles.tile([P, HC, H], F32)
    w2_r = w2.rearrange("(hb p) k -> p hb k", p=P)
    # the gpsimd queue is idle after the two gathers -- give it half of W2
    # so both weight matrices land ~2us earlier
    nc.gpsimd.dma_start(out=w2_sb[:, 0:2], in_=w2_r[:, 0:2])
    nc.scalar.dma_start(out=w2_sb[:, 2:4], in_=w2_r[:, 2:4])
    # fp16 copies of the weights: fp32 matmuls lower to two PE passes, so
    # halving the projection dtype halves the critical-path PE work.  The
    # casts run on the otherwise-idle prologue DVE, one per DMA half.
    w1_16 = singles.tile([P, HC, H], F16)
    w2_16 = singles.tile([P, HC, H], F16)
    for hb in range(HC):
        nc.scalar.copy(out=w1_16[:, hb], in_=w1_sb[:, hb])
    for hb in range(HC):
        nc.scalar.copy(out=w2_16[:, hb], in_=w2_sb[:, hb])

    # permuted-column mask bias row: 0 for u < S', -60000 beyond (-1e9 like
    # the reference would overflow fp16; any bias <= -1e3 gives
    # exp(score + bias) == 0.0 exactly in fp32, matching the reference).
    # Built on DVE: the mask matmul is the PSUM start=True writer that every
    # V-matmul waits on, so it must not sit behind slow GPSIMD ops.
    mask_bias = singles.tile([1, TE], F16)
    nc.vector.tensor_scalar(out=mask_bias, in0=iota_f[0:1, :],
                            scalar1=sprime_sb[0:1, 0:1], scalar2=-6e4,
                            op0=mybir.AluOpType.is_ge, op1=mult)

    # ---------------- transposes + projections ----------------
    # dec chain first: dec_T [h, t] then dw_T [k, t] (the X-build scalars),
    # then the enc chain: enc_T [h, u] and ew_T [k, u].
    dec_T = singles.tile([P, HC, TD], F16)
    for hb in range(HC):
        pt = psum.tile([P, P], F32, tag="tr")
        nc.tensor.transpose(pt, dec_sb[:, hb * P:(hb + 1) * P], ident)
        nc.vector.tensor_copy(out=dec_T[:, hb, :], in_=pt)
    enc_T = singles.tile([P, HC, S], F16)  # [p, hb, u] = enc[idx[u], hb*128+p]
    for (u0, ulen) in uchunks:
        esrc = enc_u1 if u0 == 0 else enc_u2
        for hb in range(HC):
            pt = psum.tile([P, P], F32, tag="tr")
            nc.tensor.transpose(pt[:, :ulen], esrc[:ulen, hb * P:(hb + 1) * P],
                                ident[:ulen, :ulen])
            nc.vector.tensor_copy(out=enc_T[:, hb, u0:u0 + ulen], in_=pt[:, :ulen])
    ew_T = singles.tile([P, HC, S], F16)  # [p, kc, u] = ew[idx[u], kc*128+p]
    for kc in range(HC):
        pew = psum.tile([P, S], F32, tag="proj")
        for hb in range(HC):
            nc.tensor.matmul(pew, lhsT=w1_16[:, hb, kc * P:(kc + 1) * P],
                             rhs=enc_T[:, hb, :], start=(hb == 0), stop=(hb == HC - 1))
        (nc.vector.tensor_copy(out=ew_T[:, kc, :], in_=pew) if kc % 2 == 0
         else nc.scalar.copy(out=ew_T[:, kc, :], in_=pew))
    dw_T = singles.tile([P, HC, TD], F32)
    for kc in range(HC):
        pdw = psum.tile([P, TD], F32, tag="proj")
        for hb in range(HC):
            nc.tensor.matmul(pdw, lhsT=w2_16[:, hb, kc * P:(kc + 1) * P],
                             rhs=dec_T[:, hb, :], start=(hb == 0), stop=(hb == HC - 1))
        (nc.vector.tensor_copy(out=dw_T[:, kc, :], in_=pdw) if kc % 2 == 0
         else nc.scalar.copy(out=dw_T[:, kc, :], in_=pdw))

    # ---------------- derived constants ----------------
    # Built on the otherwise-idle GPSIMD engine and emitted after the
    # projections so they never head-of-line block the in-order DVE stream
    # (they are only needed once the first V-matmuls run, ~20us later).
    # V32[:, c, j, :] is a [128, 32] one-hot-column stationary operand:
    # column j holds V chunk c, all other columns zero.  A matmul with it
    # accumulates V_c . Y into score row (strip_base + j) while adding zero
    # to the other 31 rows of the strip (PSUM out must be 32-row aligned).
    v32 = singles.tile([P, HC, 32, 32], F16)
    _v32ms = nc.gpsimd.memset(v32, 0.0)
    # keep the dep-free memset from being hoisted in front of the gather
    # dispatches on the in-order gpsimd engine
    bass._add_dep_helper(_v32ms.ins, _gather_last.ins, sync=False,
                         reason="order v32 memset after enc gather dispatch")
    for c in range(HC):
        v32c = v32[:, c]
        diag = bass.AP(tensor=v32c.tensor, offset=v32c.offset,
                       ap=[v32c.ap[0], [33, 32]])
        nc.gpsimd.tensor_scalar_add(out=diag, in0=diag, scalar1=v_sb32[:, c:c + 1])

    # scatter matrix G[u, s] = 1 iff idx[u] == s (padding rows hit s=0 but
    # carry exactly-0 attention, so they contribute nothing)
    idx_f = singles.tile([P, 2], F32)
    for (u0, ulen) in uchunks:
        ci = u0 // P
        nc.gpsimd.tensor_copy(out=idx_f[:ulen, ci:ci + 1],
                              in_=idx_sb[:ulen, ci:ci + 1])
    G = singles.tile([P, 2, TE], F32)
    for (u0, ulen) in uchunks:
        nc.gpsimd.tensor_scalar(out=G[:ulen, u0 // P, :], in0=iota_f[:ulen, :],
                                scalar1=idx_f[:ulen, u0 // P:u0 // P + 1], scalar2=None,
                                op0=mybir.AluOpType.is_equal)

    # ---------------- score accumulation ----------------
    score_ps = score_pool.tile([P, TE], F32)  # [t, u]
    # mask bias broadcast into every row t: ones[1,128].T @ mask_bias[1,256]
    nc.tensor.matmul(score_ps, lhsT=ones_row, rhs=mask_bias, start=True, stop=False,
                     skip_group_check=True)

    # Group-size schedule: small first groups let the ACT stream start as
    # soon as possible (less exposed X-build latency); small last groups
    # shorten the final tanh->V-matmul burst before the softmax.
    group_sizes = [2, 6] + [TG] * 14 + [4, 2, 2]
    assert sum(group_sizes) == TD
    t0g = 0
    for gi, tg in enumerate(group_sizes):
        X = xpool.tile([P, TG * HC * S], F16)
        Y = ypool.tile([P, TG * HC * S], F16)
        # Steady-state groups compute their last t as four fused bias-tanh
        # activations on ScalarE (no X traffic, no DVE work): this balances
        # DVE's 2x-mode add rate against ACT's tanh rate.  The first two
        # groups stay pure-DVE so no extra ACT instructions sit in front of
        # the first big tanh (measured prologue regression otherwise).
        fused = (2 <= gi <= 15)
        n_dve_t = tg - 1 if fused else tg
        for tl in range(n_dve_t):
            t = t0g + tl
            for c in range(HC):
                j = tl * HC + c
                # ~1/16 of the adds go to ScalarE (as Identity+bias
                # activations) to keep DVE under the ACT tanh rate
                if not fused and j % 16 == 15:
                    nc.scalar.add(out=X[:, j * S:(j + 1) * S],
                                  in_=ew_T[:, c, :], add=dw_T[:, c, t:t + 1])
                else:
                    nc.vector.tensor_scalar(out=X[:, j * S:(j + 1) * S],
                                            in0=ew_T[:, c, :],
                                            scalar1=dw_T[:, c, t:t + 1],
                                            scalar2=None, op0=add)
        if fused:
            t = t0g + tg - 1
            for c in range(HC):
                j = (tg - 1) * HC + c
                nc.scalar.activation(out=Y[:, j * S:(j + 1) * S],
                                     in_=ew_T[:, c, :],
                                     func=mybir.ActivationFunctionType.Tanh,
                                     bias=dw_T[:, c, t:t + 1], scale=1.0)
        nc.scalar.activation(out=Y[:, :n_dve_t * HC * S],
                             in_=X[:, :n_dve_t * HC * S],
                             func=mybir.ActivationFunctionType.Tanh)
        for tl in range(tg):
            t = t0g + tl
            strip = (t // 32) * 32
            jj = t % 32
            for c in range(HC):
                j = tl * HC + c
                last = (t == TD - 1) and (c == HC - 1)
                nc.tensor.matmul(score_ps[strip:strip + 32, 0:S],
                                 lhsT=v32[:, c, jj, :],
                                 rhs=Y[:, j * S:(j + 1) * S], start=False, stop=last,
                                 skip_group_check=True, tile_position=(0, strip))
        t0g += tg

    # ---------------- softmax over u (no max-subtraction needed:
    # |score| <= sum|V| ~ 16, exp fits easily in fp32) ----------------
    p_sb = singles.tile([P, TE], F32)
    nc.scalar.activation(out=p_sb, in_=score_ps, func=mybir.ActivationFunctionType.Exp)
    den = singles.tile([P, 1], F32)
    nc.vector.tensor_reduce(out=den, in_=p_sb, axis=mybir.AxisListType.X, op=add)
    rec = singles.tile([P, 1], F32)
    nc.vector.reciprocal(out=rec, in_=den)
    attn_sb = singles.tile([P, TE], F32)  # permuted columns
    nc.vector.tensor_scalar(out=attn_sb, in0=p_sb, scalar1=rec[:, 0:1], scalar2=None,
                            op0=mult)

    # ---------------- context + attention scatter ----------------
    # attn_T [u, t] feeds both the context matmul (rhs = gathered enc) and
    # the scatter matmul (rhs = one-hot G) that un-permutes the columns.
    attn_T = singles.tile([P, 2, P], F32)
    for (u0, ulen) in uchunks:
        pt = psum.tile([P, P], F32, tag="tr")
        nc.tensor.transpose(pt[:ulen, :], attn_sb[:, u0:u0 + ulen], ident)
        nc.vector.tensor_copy(out=attn_T[:ulen, u0 // P, :], in_=pt[:ulen, :])
    ctx_ps = ctx_psum_pool.tile([P, H], F32)
    for ci, (u0, ulen) in enumerate(uchunks):
        src = enc_u1 if u0 == 0 else enc_u2
        nc.tensor.matmul(ctx_ps, lhsT=attn_T[:ulen, u0 // P, :], rhs=src[:ulen, :],
                         start=(ci == 0), stop=(ci == len(uchunks) - 1))
    ctx_sb = singles.tile([P, H], F32)
    nc.vector.tensor_copy(out=ctx_sb, in_=ctx_ps)
    nc.sync.dma_start(out=ctx_out, in_=ctx_sb)
    attn_ps = psum.tile([P, TE], F32, tag="proj")
    for ci, (u0, ulen) in enumerate(uchunks):
        nc.tensor.matmul(attn_ps, lhsT=attn_T[:ulen, u0 // P, :],
                         rhs=G[:ulen, u0 // P, :],
                         start=(ci == 0), stop=(ci == len(uchunks) - 1))
    attn_full = singles.tile([P, TE], F32)
    nc.vector.tensor_copy(out=attn_full, in_=attn_ps)
    nc.sync.dma_start(out=attn_out, in_=attn_full)


def build(S):
    nc = bacc.Bacc("TRN2", target_bir_lowering=False, debug=False, num_devices=B)
    enc = nc.dram_tensor("enc", (TE, H), F32, kind="ExternalInput").ap()
    dec = nc.dram_tensor("dec", (TD, H), F32, kind="ExternalInput").ap()
    idx = nc.dram_tensor("idx", (S,), I32, kind="ExternalInput").ap()
    sprime = nc.dram_tensor("sprime", (1,), F32, kind="ExternalInput").ap()
    w1 = nc.dram_tensor("w1", (H, H), F32, kind="ExternalInput").ap()
    w2 = nc.dram_tensor("w2", (H, H), F32, kind="ExternalInput").ap()
    v = nc.dram_tensor("v", (H, 1), F32, kind="ExternalInput").ap()
    ctx_out = nc.dram_tensor("ctx_out", (TD, H), F32, kind="ExternalOutput").ap()
    attn_out = nc.dram_tensor("attn_out", (TD, TE), F32, kind="ExternalOutput").ap()
    with tile.TileContext(nc) as tc:
        _attention_kernel(tc, S, enc, dec, idx, sprime, w1, w2, v, ctx_out, attn_out)
    nc.compile()
    return nc


_NC_CACHE = {}


def _get_nc(S):
    if S not in _NC_CACHE:
        _NC_CACHE[S] = build(S)
    return _NC_CACHE[S]


def make_in_maps(encoder_outputs, decoder_outputs, encoder_mask, W1, W2, V):
    enc = np.ascontiguousarray(np.asarray(encoder_outputs, dtype=np.float32))
    dec = np.ascontiguousarray(np.asarray(decoder_outputs, dtype=np.float32))
    msk = np.asarray(encoder_mask).astype(bool)
    w1 = np.ascontiguousarray(np.asarray(W1, dtype=np.float32))
    w2 = np.ascontiguousarray(np.asarray(W2, dtype=np.float32))
    v = np.ascontiguousarray(np.asarray(V, dtype=np.float32))
    us = [np.nonzero(msk[b])[0].astype(np.int32) for b in range(B)]
    max_sp = max(max((len(u) for u in us), default=1), 1)
    S = min(TE, ((max_sp + 15) // 16) * 16)
    in_maps = []
    for b in range(B):
        idx = np.zeros(S, np.int32)
        idx[:len(us[b])] = us[b]
        in_maps.append({
            "enc": enc[b], "dec": dec[b], "idx": idx,
            "sprime": np.array([len(us[b])], np.float32),
            "w1": w1, "w2": w2, "v": v,
        })
    return in_maps, S


def kernel(encoder_outputs, decoder_outputs, encoder_mask, W1, W2, V, **run_kwargs):
    in_maps, S = make_in_maps(encoder_outputs, decoder_outputs, encoder_mask, W1, W2, V)
    nc = _get_nc(S)
    res = bass_utils.run_bass_kernel_spmd(nc, in_maps, core_ids=list(range(B)),
                                          **run_kwargs)
    ctx = np.stack([res.results[b]["ctx_out"] for b in range(B)])
    attn = np.stack([res.results[b]["attn_out"] for b in range(B)])
    return ctx, attn


# revision 41
# speedup vs baseline: 1.0503x; 1.0070x over previous
"""Bahdanau additive attention kernel for Trainium2 (Bass/Tile).

Shapes (hardcoded from the problem spec):
  encoder_outputs [8, 256, 512] f32, decoder_outputs [8, 128, 512] f32,
  encoder_mask    [8, 256] bool,  W1/W2 [512, 512] f32,  V [512, 1] f32.

Sharding: data-parallel over batch B=8 across the 8 NeuronCores; the
W1/W2/V weights are replicated.  Each core computes one batch element:
  ew = enc @ W1; dw = dec @ W2
  score[t,s] = sum_h V[h] * tanh(ew[s,h] + dw[t,h]) + mask_bias[s]
  attn = softmax_s(score); ctx = attn @ enc

Key algorithmic point: masked encoder positions get attention weight
EXACTLY 0.0 in the reference (exp(score - 1e9 - max) underflows), so the
expensive tanh volume only needs the unmasked positions.  The kernel
gathers the ~50% unmasked encoder rows (indices prepared host-side as
sharding metadata, gather itself via on-device indirect DMA), runs the
whole pipeline on S_PAD <= 256 compacted positions, and scatters the
attention row back to the full 256 columns with a one-hot matmul.

Engine split per core (the Td*S_PAD*H element inner part):
  - DVE + GPSIMD: build X[h, (t,c,u)] = ew_T + dw_T[:, t] via
    per-partition-scalar adds (2x DVE mode; 1/8 of tiles on GPSIMD)
  - ACT: tanh on [128, TG*4*S_PAD] tiles (amortizes the per-instruction
    SBUF access bubble; ACT is the throughput floor of this problem)
  - PE : score accumulation via one-hot-padded V matmuls (M=32 strips),
    mask add as a K=1 broadcast matmul, projections/transposes/context,
    and the final scatter matmul
"""

from contextlib import ExitStack

import numpy as np

import concourse.bass as bass
import concourse.tile as tile
from concourse import bacc, bass_utils, mybir
from concourse._compat import with_exitstack
from concourse.masks import make_identity

B, TD, TE, H = 8, 128, 256, 512
P = 128
HC = H // P  # 4 h-chunks
TG = 8  # t's per ACT group
F32 = mybir.dt.float32
F16 = mybir.dt.float16
I32 = mybir.dt.int32


@with_exitstack
def _attention_kernel(ctx: ExitStack, tc: tile.TileContext, S, enc, dec, idx, sprime,
                      w1, w2, v, ctx_out, attn_out):
    nc = tc.nc
    add = mybir.AluOpType.add
    mult = mybir.AluOpType.mult

    # u-chunks of the compacted encoder axis (partition-dim blocks)
    uchunks = [(0, min(S, P))] + ([(P, S - P)] if S > P else [])

    singles = ctx.enter_context(tc.tile_pool(name="singles", bufs=1))
    xpool = ctx.enter_context(tc.tile_pool(name="xpool", bufs=3))
    ypool = ctx.enter_context(tc.tile_pool(name="ypool", bufs=3))
    psum = ctx.enter_context(tc.tile_pool(name="psum", bufs=3, space="PSUM"))
    score_pool = ctx.enter_context(tc.tile_pool(name="score", bufs=1, space="PSUM"))
    ctx_psum_pool = ctx.enter_context(tc.tile_pool(name="ctxp", bufs=1, space="PSUM"))

    # ---------------- input-independent constants first ----------------
    # (make_identity ends with a GPSIMD drain that would otherwise serialize
    # behind any DMA already queued on the gpsimd engine)
    ident = singles.tile([P, P], F32)
    make_identity(nc, ident)
    iota_i = singles.tile([P, TE], I32)  # 0..255 in every partition row
    nc.gpsimd.iota(iota_i, pattern=[[1, TE]], base=0, channel_multiplier=0)
    iota_f = singles.tile([P, TE], F32)
    nc.vector.tensor_copy(out=iota_f, in_=iota_i)
    ones_row = singles.tile([1, P], F16)
    nc.vector.memset(ones_row, 1.0)
    # PE warmup during the DMA wait: dummy matmuls keep the PE busy past the
    # HAM activity window so the transposes/projections run at 2.4 GHz.
    warm_sb = singles.tile([P, P], F16)
    nc.vector.memset(warm_sb, 0.0)
    warm_ps = psum.tile([P, P], F32, tag="tr")
    for _ in range(14):
        nc.tensor.matmul(warm_ps, lhsT=warm_sb, rhs=warm_sb, start=True, stop=True)

    # ---------------- loads ----------------
    # Each DMA queue sustains only ~95 GB/s, so the ~2.3MB of inputs are
    # split across the three dispatch engines (sync/scalar/gpsimd), ordered
    # by when the consumer needs them: idx/dec -> enc gather/W2 -> W1.
    idx_sb = singles.tile([P, 2], I32)  # column ci holds idx[ci*128 + p]
    idx_r = idx.rearrange("(u o) -> u o", o=1)
    for (u0, ulen) in uchunks:
        nc.scalar.dma_start(out=idx_sb[:ulen, u0 // P:u0 // P + 1],
                            in_=idx_r[u0:u0 + ulen])
    sprime_sb = singles.tile([1, 1], F32)
    nc.scalar.dma_start(out=sprime_sb, in_=sprime.rearrange("(u o) -> u o", o=1))
    v_sb32 = singles.tile([P, HC], F32)  # [p, c] = V[c*128+p, 0]
    nc.scalar.dma_start(out=v_sb32, in_=v.rearrange("(c p) o -> p (c o)", p=P))
    dec_sb = singles.tile([P, H], F32)
    nc.sync.dma_start(out=dec_sb, in_=dec)
    # gathered encoder rows: enc_u[j] = enc[idx[j]]
    enc_u1 = singles.tile([P, H], F32)
    _gather_last = nc.gpsimd.indirect_dma_start(
        out=enc_u1[:uchunks[0][1], :], out_offset=None, in_=enc[:],
        in_offset=bass.IndirectOffsetOnAxis(ap=idx_sb[:uchunks[0][1], 0:1], axis=0))
    if len(uchunks) > 1:
        enc_u2 = singles.tile([P, H], F32)
        _gather_last = nc.gpsimd.indirect_dma_start(
            out=enc_u2[:uchunks[1][1], :], out_offset=None, in_=enc[:],
            in_offset=bass.IndirectOffsetOnAxis(
                ap=idx_sb[:uchunks[1][1], 1:2], axis=0))
    w1_sb = singles.tile([P, HC, H], F32)  # [p, hb, k] = W1[hb*128+p, k]
    w1_r = w1.rearrange("(hb p) k -> p hb k", p=P)
    nc.sync.dma_start(out=w1_sb[:, 0:2], in_=w1_r[:, 0:2])
    nc.scalar.dma_start(out=w1_sb[:, 2:4], in_=w1_r[:, 2:4])
    w2_sb = singles.tile([P, HC, H], F32)
    w2_r = w2.rearrange("(hb p) k -> p hb k", p=P)
    # the gpsimd queue is idle after the two gathers -- give it half of W2
    # so both weight matrices land ~2us earlier
    nc.gpsimd.dma_start(out=w2_sb[:, 0:2], in_=w2_r[:, 0:2])
    nc.scalar.dma_start(out=w2_sb[:, 2:4], in_=w2_r[:, 2:4])
    # fp16 copies of the weights: fp32 matmuls lower to two PE passes, so
    # halving the projection dtype halves the critical-path PE work.  The
    # casts run on the otherwise-idle prologue DVE, one per DMA half.
    w1_16 = singles.tile([P, HC, H], F16)
    w2_16 = singles.tile([P, HC, H], F16)
    for hb in range(HC):
        nc.scalar.copy(out=w1_16[:, hb], in_=w1_sb[:, hb])
    for hb in range(HC):
        nc.scalar.copy(out=w2_16[:, hb], in_=w2_sb[:, hb])

    # permuted-column mask bias row: 0 for u < S', -60000 beyond (-1e9 like
    # the reference would overflow fp16; any bias <= -1e3 gives
    # exp(score + bias) == 0.0 exactly in fp32, matching the reference).
    # Built on DVE: the mask matmul is the PSUM start=True writer that every
    # V-matmul waits on, so it must not sit behind slow GPSIMD ops.
    mask_bias = singles.tile([1, TE], F16)
    nc.vector.tensor_scalar(out=mask_bias, in0=iota_f[0:1, :],
                            scalar1=sprime_sb[0:1, 0:1], scalar2=-6e4,
                            op0=mybir.AluOpType.is_ge, op1=mult)

    # ---------------- transposes + projections ----------------
    # dec chain first: dec_T [h, t] then dw_T [k, t] (the X-build scalars),
    # then the enc chain: enc_T [h, u] and ew_T [k, u].
    dec_T = singles.tile([P, HC, TD], F16)
    for hb in range(HC):
        pt = psum.tile([P, P], F32, tag="tr")
        nc.tensor.transpose(pt, dec_sb[:, hb * P:(hb + 1) * P], ident)
        nc.vector.tensor_copy(out=dec_T[:, hb, :], in_=pt)
    enc_T = singles.tile([P, HC, S], F16)  # [p, hb, u] = enc[idx[u], hb*128+p]
    for (u0, ulen) in uchunks:
        esrc = enc_u1 if u0 == 0 else enc_u2
        for hb in range(HC):
            pt = psum.tile([P, P], F32, tag="tr")
            nc.tensor.transpose(pt[:, :ulen], esrc[:ulen, hb * P:(hb + 1) * P],
                                ident[:ulen, :ulen])
            nc.vector.tensor_copy(out=enc_T[:, hb, u0:u0 + ulen], in_=pt[:, :ulen])
    ew_T = singles.tile([P, HC, S], F16)  # [p, kc, u] = ew[idx[u], kc*128+p]
    for kc in range(HC):
        pew = psum.tile([P, S], F32, tag="proj")
        for hb in range(HC):
            nc.tensor.matmul(pew, lhsT=w1_16[:, hb, kc * P:(kc + 1) * P],
                             rhs=enc_T[:, hb, :], start=(hb == 0), stop=(hb == HC - 1))
        (nc.vector.tensor_copy(out=ew_T[:, kc, :], in_=pew) if kc % 2 == 0
         else nc.scalar.copy(out=ew_T[:, kc, :], in_=pew))
    dw_T = singles.tile([P, HC, TD], F32)
    for kc in range(HC):
        pdw = psum.tile([P, TD], F32, tag="proj")
        for hb in range(HC):
            nc.tensor.matmul(pdw, lhsT=w2_16[:, hb, kc * P:(kc + 1) * P],
                             rhs=dec_T[:, hb, :], start=(hb == 0), stop=(hb == HC - 1))
        (nc.vector.tensor_copy(out=dw_T[:, kc, :], in_=pdw) if kc % 2 == 0
         else nc.scalar.copy(out=dw_T[:, kc, :], in_=pdw))

    # ---------------- derived constants ----------------
    # Built on the otherwise-idle GPSIMD engine and emitted after the
    # projections so they never head-of-line block the in-order DVE stream
    # (they are only needed once the first V-matmuls run, ~20us later).
    # V32[:, c, j, :] is a [128, 32] one-hot-column stationary operand:
    # column j holds V chunk c, all other columns zero.  A matmul with it
    # accumulates V_c . Y into score row (strip_base + j) while adding zero
    # to the other 31 rows of the strip (PSUM out must be 32-row aligned).
    v32 = singles.tile([P, HC, 32, 32], F16)
    _v32ms = nc.gpsimd.memset(v32, 0.0)
    # keep the dep-free memset from being hoisted in front of the gather
    # dispatches on the in-order gpsimd engine
    bass._add_dep_helper(_v32ms.ins, _gather_last.ins, sync=False,
                         reason="order v32 memset after enc gather dispatch")
    for c in range(HC):
        v32c = v32[:, c]
        diag = bass.AP(tensor=v32c.tensor, offset=v32c.offset,
                       ap=[v32c.ap[0], [33, 32]])
        nc.gpsimd.tensor_scalar_add(out=diag, in0=diag, scalar1=v_sb32[:, c:c + 1])

    # scatter matrix G[u, s] = 1 iff idx[u] == s (padding rows hit s=0 but
    # carry exactly-0 attention, so they contribute nothing)
    idx_f = singles.tile([P, 2], F32)
    for (u0, ulen) in uchunks:
        ci = u0 // P
        nc.gpsimd.tensor_copy(out=idx_f[:ulen, ci:ci + 1],
                              in_=idx_sb[:ulen, ci:ci + 1])
    G = singles.tile([P, 2, TE], F32)
    for (u0, ulen) in uchunks:
        nc.gpsimd.tensor_scalar(out=G[:ulen, u0 // P, :], in0=iota_f[:ulen, :],
                                scalar1=idx_f[:ulen, u0 // P:u0 // P + 1], scalar2=None,
                                op0=mybir.AluOpType.is_equal)

    # ---------------- score accumulation ----------------
    score_ps = score_pool.tile([P, TE], F32)  # [t, u]
    # mask bias broadcast into every row t: ones[1,128].T @ mask_bias[1,256]
    nc.tensor.matmul(score_ps, lhsT=ones_row, rhs=mask_bias, start=True, stop=False,
                     skip_group_check=True)

    # Group-size schedule: small first groups let the ACT stream start as
    # soon as possible (less exposed X-build latency); small last groups
    # shorten the final tanh->V-matmul burst before the softmax.
    group_sizes = [2, 6] + [TG] * 14 + [4, 2, 2]
    assert sum(group_sizes) == TD
    t0g = 0
    for gi, tg in enumerate(group_sizes):
        X = xpool.tile([P, TG * HC * S], F16)
        Y = ypool.tile([P, TG * HC * S], F16)
        # Steady-state groups compute their last t as four fused bias-tanh
        # activations on ScalarE (no X traffic, no DVE work): this balances
        # DVE's 2x-mode add rate against ACT's tanh rate.  The first two
        # groups stay pure-DVE so no extra ACT instructions sit in front of
        # the first big tanh (measured prologue regression otherwise).
        # fused-slice count per group size (d* = (0.305*4*tg+0.185)/0.352
        # balances DVE adds against ACT tanh); gi=0 stays pure-DVE so no
        # ACT instructions sit in front of the first big tanh.
        n_fused_sl = {6: 3, 4: 2}.get(tg, 4) if gi != 0 and tg >= 4 else 0
        fused = n_fused_sl > 0
        n_dve_t = tg - 1 if fused else tg
        for tl in range(n_dve_t):
            t = t0g + tl
            for c in range(HC):
                j = tl * HC + c
                # ~1/16 of the adds go to ScalarE (as Identity+bias
                # activations) to keep DVE under the ACT tanh rate
                if not fused and j % 16 == 15:
                    nc.scalar.add(out=X[:, j * S:(j + 1) * S],
                                  in_=ew_T[:, c, :], add=dw_T[:, c, t:t + 1])
                else:
                    nc.vector.tensor_scalar(out=X[:, j * S:(j + 1) * S],
                                            in0=ew_T[:, c, :],
                                            scalar1=dw_T[:, c, t:t + 1],
                                            scalar2=None, op0=add)
        if fused:
            t = t0g + tg - 1
            for c in range(HC):
                j = (tg - 1) * HC + c
                if c < HC - n_fused_sl:
                    nc.vector.tensor_scalar(out=X[:, j * S:(j + 1) * S],
                                            in0=ew_T[:, c, :],
                                            scalar1=dw_T[:, c, t:t + 1],
                                            scalar2=None, op0=add)
                    nc.scalar.activation(out=Y[:, j * S:(j + 1) * S],
                                         in_=X[:, j * S:(j + 1) * S],
                                         func=mybir.ActivationFunctionType.Tanh)
                else:
                    nc.scalar.activation(out=Y[:, j * S:(j + 1) * S],
                                         in_=ew_T[:, c, :],
                                         func=mybir.ActivationFunctionType.Tanh,
                                         bias=dw_T[:, c, t:t + 1], scale=1.0)
        nc.scalar.activation(out=Y[:, :n_dve_t * HC * S],
                             in_=X[:, :n_dve_t * HC * S],
                             func=mybir.ActivationFunctionType.Tanh)
        for tl in range(tg):
            t = t0g + tl
            strip = (t // 32) * 32
            jj = t % 32
            for c in range(HC):
                j = tl * HC + c
                last = (t == TD - 1) and (c == HC - 1)
                nc.tensor.matmul(score_ps[strip:strip + 32, 0:S],
                                 lhsT=v32[:, c, jj, :],
                                 rhs=Y[:, j * S:(j + 1) * S], start=False, stop=last,
                                 skip_group_check=True, tile_position=(0, strip))
        t0g += tg

    # ---------------- softmax over u (no max-subtraction needed:
    # |score| <= sum|V| ~ 16, exp fits easily in fp32) ----------------
    p_sb = singles.tile([P, TE], F32)
    nc.scalar.activation(out=p_sb, in_=score_ps, func=mybir.ActivationFunctionType.Exp)
    den = singles.tile([P, 1], F32)
    nc.vector.tensor_reduce(out=den, in_=p_sb, axis=mybir.AxisListType.X, op=add)
    rec = singles.tile([P, 1], F32)
    nc.vector.reciprocal(out=rec, in_=den)
    attn_sb = singles.tile([P, TE], F32)  # permuted columns
    nc.vector.tensor_scalar(out=attn_sb, in0=p_sb, scalar1=rec[:, 0:1], scalar2=None,
                            op0=mult)

    # ---------------- context + attention scatter ----------------
    # attn_T [u, t] feeds both the context matmul (rhs = gathered enc) and
    # the scatter matmul (rhs = one-hot G) that un-permutes the columns.
    attn_T = singles.tile([P, 2, P], F32)
    for (u0, ulen) in uchunks:
        pt = psum.tile([P, P], F32, tag="tr")
        nc.tensor.transpose(pt[:ulen, :], attn_sb[:, u0:u0 + ulen], ident)
        nc.vector.tensor_copy(out=attn_T[:ulen, u0 // P, :], in_=pt[:ulen, :])
    ctx_ps = ctx_psum_pool.tile([P, H], F32)
    for ci, (u0, ulen) in enumerate(uchunks):
        src = enc_u1 if u0 == 0 else enc_u2
        nc.tensor.matmul(ctx_ps, lhsT=attn_T[:ulen, u0 // P, :], rhs=src[:ulen, :],
                         start=(ci == 0), stop=(ci == len(uchunks) - 1))
    ctx_sb = singles.tile([P, H], F32)
    nc.vector.tensor_copy(out=ctx_sb, in_=ctx_ps)
    nc.sync.dma_start(out=ctx_out, in_=ctx_sb)
    attn_ps = psum.tile([P, TE], F32, tag="proj")
    for ci, (u0, ulen) in enumerate(uchunks):
        nc.tensor.matmul(attn_ps, lhsT=attn_T[:ulen, u0 // P, :],
                         rhs=G[:ulen, u0 // P, :],
                         start=(ci == 0), stop=(ci == len(uchunks) - 1))
    attn_full = singles.tile([P, TE], F32)
    nc.vector.tensor_copy(out=attn_full, in_=attn_ps)
    nc.sync.dma_start(out=attn_out, in_=attn_full)


def build(S):
    nc = bacc.Bacc("TRN2", target_bir_lowering=False, debug=False, num_devices=B)
    enc = nc.dram_tensor("enc", (TE, H), F32, kind="ExternalInput").ap()
    dec = nc.dram_tensor("dec", (TD, H), F32, kind="ExternalInput").ap()
    idx = nc.dram_tensor("idx", (S,), I32, kind="ExternalInput").ap()
    sprime = nc.dram_tensor("sprime", (1,), F32, kind="ExternalInput").ap()
    w1 = nc.dram_tensor("w1", (H, H), F32, kind="ExternalInput").ap()
    w2 = nc.dram_tensor("w2", (H, H), F32, kind="ExternalInput").ap()
    v = nc.dram_tensor("v", (H, 1), F32, kind="ExternalInput").ap()
    ctx_out = nc.dram_tensor("ctx_out", (TD, H), F32, kind="ExternalOutput").ap()
    attn_out = nc.dram_tensor("attn_out", (TD, TE), F32, kind="ExternalOutput").ap()
    with tile.TileContext(nc) as tc:
        _attention_kernel(tc, S, enc, dec, idx, sprime, w1, w2, v, ctx_out, attn_out)
    nc.compile()
    return nc


_NC_CACHE = {}


def _get_nc(S):
    if S not in _NC_CACHE:
        _NC_CACHE[S] = build(S)
    return _NC_CACHE[S]


def make_in_maps(encoder_outputs, decoder_outputs, encoder_mask, W1, W2, V):
    enc = np.ascontiguousarray(np.asarray(encoder_outputs, dtype=np.float32))
    dec = np.ascontiguousarray(np.asarray(decoder_outputs, dtype=np.float32))
    msk = np.asarray(encoder_mask).astype(bool)
    w1 = np.ascontiguousarray(np.asarray(W1, dtype=np.float32))
    w2 = np.ascontiguousarray(np.asarray(W2, dtype=np.float32))
    v = np.ascontiguousarray(np.asarray(V, dtype=np.float32))
    us = [np.nonzero(msk[b])[0].astype(np.int32) for b in range(B)]
    max_sp = max(max((len(u) for u in us), default=1), 1)
    S = min(TE, ((max_sp + 15) // 16) * 16)
    in_maps = []
    for b in range(B):
        idx = np.zeros(S, np.int32)
        idx[:len(us[b])] = us[b]
        in_maps.append({
            "enc": enc[b], "dec": dec[b], "idx": idx,
            "sprime": np.array([len(us[b])], np.float32),
            "w1": w1, "w2": w2, "v": v,
        })
    return in_maps, S


def kernel(encoder_outputs, decoder_outputs, encoder_mask, W1, W2, V, **run_kwargs):
    in_maps, S = make_in_maps(encoder_outputs, decoder_outputs, encoder_mask, W1, W2, V)
    nc = _get_nc(S)
    res = bass_utils.run_bass_kernel_spmd(nc, in_maps, core_ids=list(range(B)),
                                          **run_kwargs)
    ctx = np.stack([res.results[b]["ctx_out"] for b in range(B)])
    attn = np.stack([res.results[b]["attn_out"] for b in range(B)])
    return ctx, attn


# revision 42
# speedup vs baseline: 1.0508x; 1.0005x over previous
"""Bahdanau additive attention kernel for Trainium2 (Bass/Tile).

Shapes (hardcoded from the problem spec):
  encoder_outputs [8, 256, 512] f32, decoder_outputs [8, 128, 512] f32,
  encoder_mask    [8, 256] bool,  W1/W2 [512, 512] f32,  V [512, 1] f32.

Sharding: data-parallel over batch B=8 across the 8 NeuronCores; the
W1/W2/V weights are replicated.  Each core computes one batch element:
  ew = enc @ W1; dw = dec @ W2
  score[t,s] = sum_h V[h] * tanh(ew[s,h] + dw[t,h]) + mask_bias[s]
  attn = softmax_s(score); ctx = attn @ enc

Key algorithmic point: masked encoder positions get attention weight
EXACTLY 0.0 in the reference (exp(score - 1e9 - max) underflows), so the
expensive tanh volume only needs the unmasked positions.  The kernel
gathers the ~50% unmasked encoder rows (indices prepared host-side as
sharding metadata, gather itself via on-device indirect DMA), runs the
whole pipeline on S_PAD <= 256 compacted positions, and scatters the
attention row back to the full 256 columns with a one-hot matmul.

Engine split per core (the Td*S_PAD*H element inner part):
  - DVE + GPSIMD: build X[h, (t,c,u)] = ew_T + dw_T[:, t] via
    per-partition-scalar adds (2x DVE mode; 1/8 of tiles on GPSIMD)
  - ACT: tanh on [128, TG*4*S_PAD] tiles (amortizes the per-instruction
    SBUF access bubble; ACT is the throughput floor of this problem)
  - PE : score accumulation via one-hot-padded V matmuls (M=32 strips),
    mask add as a K=1 broadcast matmul, projections/transposes/context,
    and the final scatter matmul
"""

from contextlib import ExitStack

import numpy as np

import concourse.bass as bass
import concourse.tile as tile
from concourse import bacc, bass_utils, mybir
from concourse._compat import with_exitstack
from concourse.masks import make_identity

B, TD, TE, H = 8, 128, 256, 512
P = 128
HC = H // P  # 4 h-chunks
TG = 8  # t's per ACT group
F32 = mybir.dt.float32
F16 = mybir.dt.float16
I32 = mybir.dt.int32


@with_exitstack
def _attention_kernel(ctx: ExitStack, tc: tile.TileContext, S, enc, dec, idx, sprime,
                      w1, w2, v, ctx_out, attn_out):
    nc = tc.nc
    add = mybir.AluOpType.add
    mult = mybir.AluOpType.mult

    # u-chunks of the compacted encoder axis (partition-dim blocks)
    uchunks = [(0, min(S, P))] + ([(P, S - P)] if S > P else [])

    singles = ctx.enter_context(tc.tile_pool(name="singles", bufs=1))
    xpool = ctx.enter_context(tc.tile_pool(name="xpool", bufs=4))
    ypool = ctx.enter_context(tc.tile_pool(name="ypool", bufs=4))
    psum = ctx.enter_context(tc.tile_pool(name="psum", bufs=3, space="PSUM"))
    score_pool = ctx.enter_context(tc.tile_pool(name="score", bufs=1, space="PSUM"))
    ctx_psum_pool = ctx.enter_context(tc.tile_pool(name="ctxp", bufs=1, space="PSUM"))

    # ---------------- input-independent constants first ----------------
    # (make_identity ends with a GPSIMD drain that would otherwise serialize
    # behind any DMA already queued on the gpsimd engine)
    ident = singles.tile([P, P], F32)
    make_identity(nc, ident)
    iota_i = singles.tile([P, TE], I32)  # 0..255 in every partition row
    nc.gpsimd.iota(iota_i, pattern=[[1, TE]], base=0, channel_multiplier=0)
    iota_f = singles.tile([P, TE], F32)
    nc.vector.tensor_copy(out=iota_f, in_=iota_i)
    ones_row = singles.tile([1, P], F16)
    nc.vector.memset(ones_row, 1.0)
    # PE warmup during the DMA wait: dummy matmuls keep the PE busy past the
    # HAM activity window so the transposes/projections run at 2.4 GHz.
    warm_sb = singles.tile([P, P], F16)
    nc.vector.memset(warm_sb, 0.0)
    warm_ps = psum.tile([P, P], F32, tag="tr")
    for _ in range(14):
        nc.tensor.matmul(warm_ps, lhsT=warm_sb, rhs=warm_sb, start=True, stop=True)

    # ---------------- loads ----------------
    # Each DMA queue sustains only ~95 GB/s, so the ~2.3MB of inputs are
    # split across the three dispatch engines (sync/scalar/gpsimd), ordered
    # by when the consumer needs them: idx/dec -> enc gather/W2 -> W1.
    idx_sb = singles.tile([P, 2], I32)  # column ci holds idx[ci*128 + p]
    idx_r = idx.rearrange("(u o) -> u o", o=1)
    for (u0, ulen) in uchunks:
        nc.scalar.dma_start(out=idx_sb[:ulen, u0 // P:u0 // P + 1],
                            in_=idx_r[u0:u0 + ulen])
    sprime_sb = singles.tile([1, 1], F32)
    nc.scalar.dma_start(out=sprime_sb, in_=sprime.rearrange("(u o) -> u o", o=1))
    v_sb32 = singles.tile([P, HC], F32)  # [p, c] = V[c*128+p, 0]
    nc.scalar.dma_start(out=v_sb32, in_=v.rearrange("(c p) o -> p (c o)", p=P))
    dec_sb = singles.tile([P, H], F32)
    nc.sync.dma_start(out=dec_sb, in_=dec)
    # gathered encoder rows: enc_u[j] = enc[idx[j]]
    enc_u1 = singles.tile([P, H], F32)
    _gather_last = nc.gpsimd.indirect_dma_start(
        out=enc_u1[:uchunks[0][1], :], out_offset=None, in_=enc[:],
        in_offset=bass.IndirectOffsetOnAxis(ap=idx_sb[:uchunks[0][1], 0:1], axis=0))
    if len(uchunks) > 1:
        enc_u2 = singles.tile([P, H], F32)
        _gather_last = nc.gpsimd.indirect_dma_start(
            out=enc_u2[:uchunks[1][1], :], out_offset=None, in_=enc[:],
            in_offset=bass.IndirectOffsetOnAxis(
                ap=idx_sb[:uchunks[1][1], 1:2], axis=0))
    w1_sb = singles.tile([P, HC, H], F32)  # [p, hb, k] = W1[hb*128+p, k]
    w1_r = w1.rearrange("(hb p) k -> p hb k", p=P)
    nc.sync.dma_start(out=w1_sb[:, 0:2], in_=w1_r[:, 0:2])
    nc.scalar.dma_start(out=w1_sb[:, 2:4], in_=w1_r[:, 2:4])
    w2_sb = singles.tile([P, HC, H], F32)
    w2_r = w2.rearrange("(hb p) k -> p hb k", p=P)
    # the gpsimd queue is idle after the two gathers -- give it half of W2
    # so both weight matrices land ~2us earlier
    nc.gpsimd.dma_start(out=w2_sb[:, 0:2], in_=w2_r[:, 0:2])
    nc.scalar.dma_start(out=w2_sb[:, 2:4], in_=w2_r[:, 2:4])
    # fp16 copies of the weights: fp32 matmuls lower to two PE passes, so
    # halving the projection dtype halves the critical-path PE work.  The
    # casts run on the otherwise-idle prologue DVE, one per DMA half.
    w1_16 = singles.tile([P, HC, H], F16)
    w2_16 = singles.tile([P, HC, H], F16)
    for hb in range(HC):
        nc.scalar.copy(out=w1_16[:, hb], in_=w1_sb[:, hb])
    for hb in range(HC):
        nc.scalar.copy(out=w2_16[:, hb], in_=w2_sb[:, hb])

    # permuted-column mask bias row: 0 for u < S', -60000 beyond (-1e9 like
    # the reference would overflow fp16; any bias <= -1e3 gives
    # exp(score + bias) == 0.0 exactly in fp32, matching the reference).
    # Built on DVE: the mask matmul is the PSUM start=True writer that every
    # V-matmul waits on, so it must not sit behind slow GPSIMD ops.
    mask_bias = singles.tile([1, TE], F16)
    nc.vector.tensor_scalar(out=mask_bias, in0=iota_f[0:1, :],
                            scalar1=sprime_sb[0:1, 0:1], scalar2=-6e4,
                            op0=mybir.AluOpType.is_ge, op1=mult)

    # ---------------- transposes + projections ----------------
    # dec chain first: dec_T [h, t] then dw_T [k, t] (the X-build scalars),
    # then the enc chain: enc_T [h, u] and ew_T [k, u].
    dec_T = singles.tile([P, HC, TD], F16)
    for hb in range(HC):
        pt = psum.tile([P, P], F32, tag="tr")
        nc.tensor.transpose(pt, dec_sb[:, hb * P:(hb + 1) * P], ident)
        nc.vector.tensor_copy(out=dec_T[:, hb, :], in_=pt)
    enc_T = singles.tile([P, HC, S], F16)  # [p, hb, u] = enc[idx[u], hb*128+p]
    for (u0, ulen) in uchunks:
        esrc = enc_u1 if u0 == 0 else enc_u2
        for hb in range(HC):
            pt = psum.tile([P, P], F32, tag="tr")
            nc.tensor.transpose(pt[:, :ulen], esrc[:ulen, hb * P:(hb + 1) * P],
                                ident[:ulen, :ulen])
            nc.vector.tensor_copy(out=enc_T[:, hb, u0:u0 + ulen], in_=pt[:, :ulen])
    ew_T = singles.tile([P, HC, S], F16)  # [p, kc, u] = ew[idx[u], kc*128+p]
    for kc in range(HC):
        pew = psum.tile([P, S], F32, tag="proj")
        for hb in range(HC):
            nc.tensor.matmul(pew, lhsT=w1_16[:, hb, kc * P:(kc + 1) * P],
                             rhs=enc_T[:, hb, :], start=(hb == 0), stop=(hb == HC - 1))
        (nc.vector.tensor_copy(out=ew_T[:, kc, :], in_=pew) if kc % 2 == 0
         else nc.scalar.copy(out=ew_T[:, kc, :], in_=pew))
    dw_T = singles.tile([P, HC, TD], F32)
    for kc in range(HC):
        pdw = psum.tile([P, TD], F32, tag="proj")
        for hb in range(HC):
            nc.tensor.matmul(pdw, lhsT=w2_16[:, hb, kc * P:(kc + 1) * P],
                             rhs=dec_T[:, hb, :], start=(hb == 0), stop=(hb == HC - 1))
        (nc.vector.tensor_copy(out=dw_T[:, kc, :], in_=pdw) if kc % 2 == 0
         else nc.scalar.copy(out=dw_T[:, kc, :], in_=pdw))

    # ---------------- derived constants ----------------
    # Built on the otherwise-idle GPSIMD engine and emitted after the
    # projections so they never head-of-line block the in-order DVE stream
    # (they are only needed once the first V-matmuls run, ~20us later).
    # V32[:, c, j, :] is a [128, 32] one-hot-column stationary operand:
    # column j holds V chunk c, all other columns zero.  A matmul with it
    # accumulates V_c . Y into score row (strip_base + j) while adding zero
    # to the other 31 rows of the strip (PSUM out must be 32-row aligned).
    v32 = singles.tile([P, HC, 32, 32], F16)
    _v32ms = nc.gpsimd.memset(v32, 0.0)
    # keep the dep-free memset from being hoisted in front of the gather
    # dispatches on the in-order gpsimd engine
    bass._add_dep_helper(_v32ms.ins, _gather_last.ins, sync=False,
                         reason="order v32 memset after enc gather dispatch")
    for c in range(HC):
        v32c = v32[:, c]
        diag = bass.AP(tensor=v32c.tensor, offset=v32c.offset,
                       ap=[v32c.ap[0], [33, 32]])
        nc.gpsimd.tensor_scalar_add(out=diag, in0=diag, scalar1=v_sb32[:, c:c + 1])

    # scatter matrix G[u, s] = 1 iff idx[u] == s (padding rows hit s=0 but
    # carry exactly-0 attention, so they contribute nothing)
    idx_f = singles.tile([P, 2], F32)
    for (u0, ulen) in uchunks:
        ci = u0 // P
        nc.gpsimd.tensor_copy(out=idx_f[:ulen, ci:ci + 1],
                              in_=idx_sb[:ulen, ci:ci + 1])
    G = singles.tile([P, 2, TE], F32)
    for (u0, ulen) in uchunks:
        nc.gpsimd.tensor_scalar(out=G[:ulen, u0 // P, :], in0=iota_f[:ulen, :],
                                scalar1=idx_f[:ulen, u0 // P:u0 // P + 1], scalar2=None,
                                op0=mybir.AluOpType.is_equal)

    # ---------------- score accumulation ----------------
    score_ps = score_pool.tile([P, TE], F32)  # [t, u]
    # mask bias broadcast into every row t: ones[1,128].T @ mask_bias[1,256]
    nc.tensor.matmul(score_ps, lhsT=ones_row, rhs=mask_bias, start=True, stop=False,
                     skip_group_check=True)

    # Group-size schedule: small first groups let the ACT stream start as
    # soon as possible (less exposed X-build latency); small last groups
    # shorten the final tanh->V-matmul burst before the softmax.
    group_sizes = [2, 6] + [TG] * 14 + [4, 2, 2]
    assert sum(group_sizes) == TD
    t0g = 0
    for gi, tg in enumerate(group_sizes):
        X = xpool.tile([P, TG * HC * S], F16)
        Y = ypool.tile([P, TG * HC * S], F16)
        # Steady-state groups compute their last t as four fused bias-tanh
        # activations on ScalarE (no X traffic, no DVE work): this balances
        # DVE's 2x-mode add rate against ACT's tanh rate.  The first two
        # groups stay pure-DVE so no extra ACT instructions sit in front of
        # the first big tanh (measured prologue regression otherwise).
        # fused-slice count per group size (d* = (0.305*4*tg+0.185)/0.352
        # balances DVE adds against ACT tanh); gi=0 stays pure-DVE so no
        # ACT instructions sit in front of the first big tanh.
        n_fused_sl = {6: 3, 4: 2}.get(tg, 4) if gi != 0 and tg >= 4 else 0
        fused = n_fused_sl > 0
        n_dve_t = tg - 1 if fused else tg
        for tl in range(n_dve_t):
            t = t0g + tl
            for c in range(HC):
                j = tl * HC + c
                # ~1/16 of the adds go to ScalarE (as Identity+bias
                # activations) to keep DVE under the ACT tanh rate
                if not fused and j % 16 == 15:
                    nc.scalar.add(out=X[:, j * S:(j + 1) * S],
                                  in_=ew_T[:, c, :], add=dw_T[:, c, t:t + 1])
                else:
                    nc.vector.tensor_scalar(out=X[:, j * S:(j + 1) * S],
                                            in0=ew_T[:, c, :],
                                            scalar1=dw_T[:, c, t:t + 1],
                                            scalar2=None, op0=add)
        if fused:
            t = t0g + tg - 1
            for c in range(HC):
                j = (tg - 1) * HC + c
                if c < HC - n_fused_sl:
                    nc.vector.tensor_scalar(out=X[:, j * S:(j + 1) * S],
                                            in0=ew_T[:, c, :],
                                            scalar1=dw_T[:, c, t:t + 1],
                                            scalar2=None, op0=add)
                    nc.scalar.activation(out=Y[:, j * S:(j + 1) * S],
                                         in_=X[:, j * S:(j + 1) * S],
                                         func=mybir.ActivationFunctionType.Tanh)
                else:
                    nc.scalar.activation(out=Y[:, j * S:(j + 1) * S],
                                         in_=ew_T[:, c, :],
                                         func=mybir.ActivationFunctionType.Tanh,
                                         bias=dw_T[:, c, t:t + 1], scale=1.0)
        nc.scalar.activation(out=Y[:, :n_dve_t * HC * S],
                             in_=X[:, :n_dve_t * HC * S],
                             func=mybir.ActivationFunctionType.Tanh)
        for tl in range(tg):
            t = t0g + tl
            strip = (t // 32) * 32
            jj = t % 32
            for c in range(HC):
                j = tl * HC + c
                last = (t == TD - 1) and (c == HC - 1)
                nc.tensor.matmul(score_ps[strip:strip + 32, 0:S],
                                 lhsT=v32[:, c, jj, :],
                                 rhs=Y[:, j * S:(j + 1) * S], start=False, stop=last,
                                 skip_group_check=True, tile_position=(0, strip))
        t0g += tg

    # ---------------- softmax over u (no max-subtraction needed:
    # |score| <= sum|V| ~ 16, exp fits easily in fp32) ----------------
    p_sb = singles.tile([P, TE], F32)
    nc.scalar.activation(out=p_sb, in_=score_ps, func=mybir.ActivationFunctionType.Exp)
    den = singles.tile([P, 1], F32)
    nc.vector.tensor_reduce(out=den, in_=p_sb, axis=mybir.AxisListType.X, op=add)
    rec = singles.tile([P, 1], F32)
    nc.vector.reciprocal(out=rec, in_=den)
    attn_sb = singles.tile([P, TE], F32)  # permuted columns
    nc.vector.tensor_scalar(out=attn_sb, in0=p_sb, scalar1=rec[:, 0:1], scalar2=None,
                            op0=mult)

    # ---------------- context + attention scatter ----------------
    # attn_T [u, t] feeds both the context matmul (rhs = gathered enc) and
    # the scatter matmul (rhs = one-hot G) that un-permutes the columns.
    attn_T = singles.tile([P, 2, P], F32)
    for (u0, ulen) in uchunks:
        pt = psum.tile([P, P], F32, tag="tr")
        nc.tensor.transpose(pt[:ulen, :], attn_sb[:, u0:u0 + ulen], ident)
        nc.vector.tensor_copy(out=attn_T[:ulen, u0 // P, :], in_=pt[:ulen, :])
    ctx_ps = ctx_psum_pool.tile([P, H], F32)
    for ci, (u0, ulen) in enumerate(uchunks):
        src = enc_u1 if u0 == 0 else enc_u2
        nc.tensor.matmul(ctx_ps, lhsT=attn_T[:ulen, u0 // P, :], rhs=src[:ulen, :],
                         start=(ci == 0), stop=(ci == len(uchunks) - 1))
    ctx_sb = singles.tile([P, H], F32)
    nc.vector.tensor_copy(out=ctx_sb, in_=ctx_ps)
    nc.sync.dma_start(out=ctx_out, in_=ctx_sb)
    attn_ps = psum.tile([P, TE], F32, tag="proj")
    for ci, (u0, ulen) in enumerate(uchunks):
        nc.tensor.matmul(attn_ps, lhsT=attn_T[:ulen, u0 // P, :],
                         rhs=G[:ulen, u0 // P, :],
                         start=(ci == 0), stop=(ci == len(uchunks) - 1))
    attn_full = singles.tile([P, TE], F32)
    nc.vector.tensor_copy(out=attn_full, in_=attn_ps)
    nc.sync.dma_start(out=attn_out, in_=attn_full)


def build(S):
    nc = bacc.Bacc("TRN2", target_bir_lowering=False, debug=False, num_devices=B)
    enc = nc.dram_tensor("enc", (TE, H), F32, kind="ExternalInput").ap()
    dec = nc.dram_tensor("dec", (TD, H), F32, kind="ExternalInput").ap()
    idx = nc.dram_tensor("idx", (S,), I32, kind="ExternalInput").ap()
    sprime = nc.dram_tensor("sprime", (1,), F32, kind="ExternalInput").ap()
    w1 = nc.dram_tensor("w1", (H, H), F32, kind="ExternalInput").ap()
    w2 = nc.dram_tensor("w2", (H, H), F32, kind="ExternalInput").ap()
    v = nc.dram_tensor("v", (H, 1), F32, kind="ExternalInput").ap()
    ctx_out = nc.dram_tensor("ctx_out", (TD, H), F32, kind="ExternalOutput").ap()
    attn_out = nc.dram_tensor("attn_out", (TD, TE), F32, kind="ExternalOutput").ap()
    with tile.TileContext(nc) as tc:
        _attention_kernel(tc, S, enc, dec, idx, sprime, w1, w2, v, ctx_out, attn_out)
    nc.compile()
    return nc


_NC_CACHE = {}


def _get_nc(S):
    if S not in _NC_CACHE:
        _NC_CACHE[S] = build(S)
    return _NC_CACHE[S]


def make_in_maps(encoder_outputs, decoder_outputs, encoder_mask, W1, W2, V):
    enc = np.ascontiguousarray(np.asarray(encoder_outputs, dtype=np.float32))
    dec = np.ascontiguousarray(np.asarray(decoder_outputs, dtype=np.float32))
    msk = np.asarray(encoder_mask).astype(bool)
    w1 = np.ascontiguousarray(np.asarray(W1, dtype=np.float32))
    w2 = np.ascontiguousarray(np.asarray(W2, dtype=np.float32))
    v = np.ascontiguousarray(np.asarray(V, dtype=np.float32))
    us = [np.nonzero(msk[b])[0].astype(np.int32) for b in range(B)]
    max_sp = max(max((len(u) for u in us), default=1), 1)
    S = min(TE, ((max_sp + 15) // 16) * 16)
    in_maps = []
    for b in range(B):
        idx = np.zeros(S, np.int32)
        idx[:len(us[b])] = us[b]
        in_maps.append({
            "enc": enc[b], "dec": dec[b], "idx": idx,
            "sprime": np.array([len(us[b])], np.float32),
            "w1": w1, "w2": w2, "v": v,
        })
    return in_maps, S


def kernel(encoder_outputs, decoder_outputs, encoder_mask, W1, W2, V, **run_kwargs):
    in_maps, S = make_in_maps(encoder_outputs, decoder_outputs, encoder_mask, W1, W2, V)
    nc = _get_nc(S)
    res = bass_utils.run_bass_kernel_spmd(nc, in_maps, core_ids=list(range(B)),
                                          **run_kwargs)
    ctx = np.stack([res.results[b]["ctx_out"] for b in range(B)])
    attn = np.stack([res.results[b]["attn_out"] for b in range(B)])
    return ctx, attn
